# revision 16
# baseline (speedup 1.0000x reference)
"""Trainium2 Bass kernel for nn_Attention_LR_65249143160949 (cross-attention block).

Sharding: 8 cores = 4 batches x 2 token-halves (1152 tokens each). Each core
computes k/v for its whole batch (cheap MQA single head, duplicated within the
pair) and q/attention/output for its own tokens. The host permutes tokens so
each core's own rows come first -> identical SPMD program, no collectives.

On-chip layout: features on partitions, tokens on the free axis (matches the
channels-first HBM layout; no input transpose). LayerNorm is folded into the
projections: q = rs_i * (x @ Wq' - mu_i * colsum(Wq')), with Wq' pre-scaled on
the host; k/v analogous (rank-1 -colsum*mu matmul accumulated into the same
psum group). Attention runs in sim^T layout (keys on partitions, query tokens
on the free axis): kT is pre-scaled by rs_j so softmax is a plain exp; the
denominator comes free as a ones-column appended to v (row 64 of the out
psum); out^T columns are normalized by a PE-broadcast reciprocal row.
Per-token LN stats come from PE matmuls (ones as one operand), never from
cross-partition vector ops.

Precision: fp32 end-to-end math except the five big matmul groups
(q/kv/sim/attn.v/out-proj), whose operands are bf16 with fp32 PSUM
accumulation. LN statistics, softmax normalization, and the residual path
stay fp32.

Two walrus quirks are handled: every TPB instruction holds at most ONE sync
wait (extras are split onto same-engine NoOps by _split_multi_waits), and
custom DVE ops are unavailable (exact reciprocal is used).
"""

import sys

import numpy as np

if "/opt/trn_rl_repo" not in sys.path:
    sys.path.insert(0, "/opt/trn_rl_repo")

C = 512          # channels
N = 2304         # tokens per batch (48*48)
NH = 1152        # tokens per core
HEADS = 8
DH = 64
CTXL = 77
CTXD = 768
JT = 19          # j tiles of 128: 18 img + 1 (ctx 0:77 | null 77 | pad)
JP = JT * 128
CH = 384         # token chunk (psum free dim)
NCH = NH // CH   # 3
KT = 4           # C / 128
EPS = 1e-5

PROFILE = False
PROFILE_DIR = None

_cached = {}


def _split_multi_waits(nc):
    """Walrus codegen supports one sync-wait per TPB instruction (the EVENTS
    struct has a single wait slot). Tile attaches several. Split the extras
    onto same-engine NoOps inserted just before each instruction."""
    import concourse.mybir as mybir

    n = 0
    for fn in nc.m.functions:
        for bb in fn.blocks:
            insts = bb.instructions
            i = 0
            while i < len(insts):
                ins = insts[i]
                si = getattr(ins, "sync_info", None)
                if si is not None and si.on_wait and len(si.on_wait) > 1:
                    waits = list(si.on_wait)
                    for w in waits[:-1]:
                        n += 1
                        nop = mybir.InstNoOp(name=f"WSPLIT-{n}", engine=ins.engine)
                        nop.sync_info = mybir.SyncInfo(on_wait=[w], on_update=[])
                        insts.insert(i, nop)
                        i += 1
                    ins.sync_info = mybir.SyncInfo(
                        on_wait=[waits[-1]], on_update=si.on_update)
                i += 1
    return n


def _build_bass():
    import concourse.bass as bass
    import concourse.mybir as mybir
    import concourse.tile as tile
    from concourse.masks import make_identity
    from contextlib import ExitStack

    F32 = mybir.dt.float32
    BF = mybir.dt.bfloat16
    AF = mybir.ActivationFunctionType
    ALU = mybir.AluOpType

    nc = bass.Bass()
    x_own = nc.declare_dram_parameter("x_own", [C, NH], F32, isOutput=False)
    x_oth = nc.declare_dram_parameter("x_oth", [C, NH], F32, isOutput=False)
    ctxt = nc.declare_dram_parameter("ctxt", [CTXL, CTXD], F32, isOutput=False)
    wq = nc.declare_dram_parameter("wq", [C, C], BF, isOutput=False)
    negcq = nc.declare_dram_parameter("negcq", [1, C], BF, isOutput=False)
    wkv = nc.declare_dram_parameter("wkv", [C, 2 * DH], BF, isOutput=False)
    ncsk = nc.declare_dram_parameter("ncsk", [1, DH], BF, isOutput=False)
    ncsv = nc.declare_dram_parameter("ncsv", [1, DH], BF, isOutput=False)
    wctx = nc.declare_dram_parameter("wctx", [CTXD, 2 * DH], F32, isOutput=False)
    bctxk = nc.declare_dram_parameter("bctxk", [DH, 1], F32, isOutput=False)
    bctxv = nc.declare_dram_parameter("bctxv", [DH, 1], F32, isOutput=False)
    nullkt = nc.declare_dram_parameter("nullkt", [DH, 1], F32, isOutput=False)
    nullv = nc.declare_dram_parameter("nullv", [DH, 1], F32, isOutput=False)
    wout = nc.declare_dram_parameter("wout", [DH, HEADS * C], BF, isOutput=False)
    outg = nc.declare_dram_parameter("outg", [128, KT], F32, isOutput=False)
    y = nc.declare_dram_parameter("y", [C, NH], F32, isOutput=True)

    with tile.TileContext(nc) as tc, ExitStack() as ctx:
        pconst = ctx.enter_context(tc.tile_pool(name="const", bufs=1))
        pbig = ctx.enter_context(tc.tile_pool(name="big", bufs=1))

        ident = pconst.tile([128, 128], F32)
        make_identity(nc, ident[:])
        ident_bf = pconst.tile([128, 128], BF)
        make_identity(nc, ident_bf[:])
        ones_col = pconst.tile([128, 1], F32)
        nc.vector.memset(ones_col[:], 1.0)
        ones_blk = pconst.tile([128, 128], F32)
        nc.vector.memset(ones_blk[:], 1.0)
        eps_col = pconst.tile([128, 1], F32)
        nc.vector.memset(eps_col[:], EPS)

        x_sb = pbig.tile([128, KT * N], F32)         # kt-major; own rows first
        x_bf = pbig.tile([128, KT * N], BF)
        qT = pbig.tile([64, HEADS * NH], BF)         # head-major blocks
        kT = pbig.tile([64, JP], BF)                 # rs-scaled keys
        v_sb = pbig.tile([128, JT * (DH + 1)], BF)   # per j-tile [128, 64+ones]
        projT = pbig.tile([128, KT * NH], F32)
        stats = pbig.tile([128, 40], F32)            # col jt: rs_j (v scaling)
        wout_sb = pbig.tile([64, HEADS * C], BF)
        outg_sb = pbig.tile([128, KT], F32)
        # per-token stat rows on partition 0: mu 0:N | rs N:2N
        # (LN2 reuses per cc: mu2 at cc*CH, rs2 at N+cc*CH, ex2 at 2N+cc*CH)
        rows = pbig.tile([1, 2 * N + NH], F32)
        rows_bf = pbig.tile([1, N], BF)
        R_RS, R_SC = N, 2 * N

        nc.sync.dma_start(wout_sb[:], wout[:, :])
        nc.sync.dma_start(outg_sb[:], outg[:, :])

        with tc.tile_pool(name="load", bufs=1) as pload, \
             tc.tile_pool(name="x2p", bufs=2) as px2, \
             tc.tile_pool(name="pss", bufs=2, space="PSUM") as pss:
            # setup psum tags: b1 [<=64,384]x2, bS [128,<=512]x4, bT [128,128]x2
            wq_sb = pload.tile([128, KT * C], BF)
            wkv_sb = pload.tile([128, KT * 2 * DH], BF)
            wctx_sb = pload.tile([128, CTXD], F32)
            negcq_sb = pload.tile([1, C], BF)
            ncsk_sb = pload.tile([1, DH], BF)
            ncsv_sb = pload.tile([1, DH], BF)
            bctxk_sb = pload.tile([DH, 1], F32)
            bctxv_sb = pload.tile([DH, 1], F32)
            vT = pload.tile([64, N], BF)
            ck_sb = pload.tile([64, CTXL], F32)
            cv_sb = pload.tile([64, CTXL + 1], F32)
            nullk_st = pload.tile([DH, 1], F32)
            nullv_st = pload.tile([DH, 1], F32)
            ctx_sb = pload.tile([CTXL, CTXD], F32)
            ctxnT = pload.tile([128, 6 * CTXL], F32)
            ex2 = pload.tile([1, N], F32)

            x_v = x_sb[:].rearrange("p (k n) -> p k n", k=KT)
            nc.sync.dma_start(x_v[:, :, 0:NH],
                              x_own[:].rearrange("(k p) n -> p k n", p=128))
            nc.sync.dma_start(x_v[:, :, NH:N],
                              x_oth[:].rearrange("(k p) n -> p k n", p=128))
            nc.sync.dma_start(wq_sb[:].rearrange("p (k n) -> p k n", k=KT),
                              wq[:].rearrange("(k p) n -> p k n", p=128))
            nc.sync.dma_start(wkv_sb[:].rearrange("p (k n) -> p k n", k=KT),
                              wkv[:].rearrange("(k p) n -> p k n", p=128))
            nc.sync.dma_start(wctx_sb[:].rearrange("p (k n) -> p k n", k=6),
                              wctx[:].rearrange("(k p) n -> p k n", p=128))
            nc.sync.dma_start(negcq_sb[:], negcq[:, :])
            nc.sync.dma_start(ncsk_sb[:], ncsk[:, :])
            nc.sync.dma_start(ncsv_sb[:], ncsv[:, :])
            nc.sync.dma_start(bctxk_sb[:], bctxk[:, :])
            nc.sync.dma_start(bctxv_sb[:], bctxv[:, :])
            nc.sync.dma_start(ctx_sb[:], ctxt[:, :])
            nc.sync.dma_start(nullk_st[:], nullkt[:, :])
            nc.sync.dma_start(nullv_st[:], nullv[:, :])

            for kt in range(KT):
                nc.vector.tensor_copy(x_bf[:, kt * N : (kt + 1) * N],
                                      x_sb[:, kt * N : (kt + 1) * N])

            # ---- LN1 stats (row form): mu, then rs = exp(-0.5 ln(var+eps)) ----
            for ch in range(6):
                sl = slice(ch * 384, (ch + 1) * 384)
                ps_r1 = pss.tile([64, 384], F32, tag="b1")
                for kt in range(KT):
                    nc.tensor.matmul(
                        ps_r1[0:1, :], ones_col[:],
                        x_sb[:, kt * N + ch * 384 : kt * N + (ch + 1) * 384],
                        start=(kt == 0), stop=(kt == KT - 1))
                nc.scalar.mul(rows[0:1, sl], ps_r1[0:1, :], 1.0 / C)
                nc.vector.tensor_copy(rows_bf[0:1, sl], rows[0:1, sl])
            for ch in range(6):
                x2 = px2.tile([128, KT * 384], F32, tag="x2")
                ps_r2 = pss.tile([64, 384], F32, tag="b1")
                for kt in range(KT):
                    xs = x_sb[:, kt * N + ch * 384 : kt * N + (ch + 1) * 384]
                    nc.vector.tensor_mul(x2[:, kt * 384 : (kt + 1) * 384], xs, xs)
                    nc.tensor.matmul(
                        ps_r2[0:1, :], ones_col[:], x2[:, kt * 384 : (kt + 1) * 384],
                        start=(kt == 0), stop=(kt == KT - 1))
                nc.scalar.mul(ex2[0:1, ch * 384 : (ch + 1) * 384],
                              ps_r2[0:1, :], 1.0 / C)
            for ch in range(6):
                a, b = R_RS + ch * 384, R_RS + (ch + 1) * 384
                mu = rows[0:1, ch * 384 : (ch + 1) * 384]
                nc.vector.tensor_mul(rows[0:1, a:b], mu, mu)
                nc.vector.tensor_sub(rows[0:1, a:b],
                                     ex2[0:1, ch * 384 : (ch + 1) * 384],
                                     rows[0:1, a:b])
                nc.scalar.activation(rows[0:1, a:b], rows[0:1, a:b], AF.Ln,
                                     bias=eps_col[0:1, :])
                nc.scalar.activation(rows[0:1, a:b], rows[0:1, a:b], AF.Exp,
                                     scale=-0.5)
            # rs as per-partition columns (v scaling)
            for jt in range(18):
                ps_c = pss.tile([128, 128], F32, tag="bT")
                nc.tensor.matmul(ps_c[:, 0:1],
                                 rows[0:1, R_RS + jt * 128 : R_RS + (jt + 1) * 128],
                                 ones_col[0:1, :])
                nc.vector.tensor_copy(stats[:, jt : jt + 1], ps_c[:, 0:1])

            # ---- context: LN (layout A, bn_stats) + k/v projection ----
            cstat = pload.tile([CTXL, 3, 6], F32)
            for sg in range(3):
                nc.vector.bn_stats(cstat[:, sg, :],
                                   ctx_sb[:, sg * 256 : (sg + 1) * 256])
            cmv = pload.tile([CTXL, 2], F32)
            nc.vector.bn_aggr(cmv[:], cstat[:])
            nc.scalar.activation(cmv[:, 1:2], cmv[:, 1:2], AF.Ln,
                                 bias=eps_col[0:CTXL, :])
            nc.scalar.activation(cmv[:, 1:2], cmv[:, 1:2], AF.Exp, scale=-0.5)
            nc.vector.tensor_scalar(
                out=ctx_sb[:], in0=ctx_sb[:],
                scalar1=cmv[:, 0:1], scalar2=cmv[:, 1:2],
                op0=ALU.subtract, op1=ALU.mult)
            for kt in range(6):
                ps_ct = pss.tile([128, 128], F32, tag="bT")
                nc.tensor.transpose(ps_ct[:, 0:CTXL],
                                    ctx_sb[:, kt * 128 : (kt + 1) * 128],
                                    ident[:CTXL, :CTXL])
                nc.vector.tensor_copy(ctxnT[:, kt * CTXL : (kt + 1) * CTXL],
                                      ps_ct[:, 0:CTXL])
            ps_ck = pss.tile([64, 384], F32, tag="b1")
            ps_cv = pss.tile([64, 384], F32, tag="b1")
            for kt in range(6):
                nc.tensor.matmul(ps_ck[:, 0:CTXL],
                                 wctx_sb[:, kt * 128 : kt * 128 + DH],
                                 ctxnT[:, kt * CTXL : (kt + 1) * CTXL],
                                 start=(kt == 0), stop=(kt == 5))
                nc.tensor.matmul(ps_cv[:, 0:CTXL],
                                 wctx_sb[:, kt * 128 + DH : (kt + 1) * 128],
                                 ctxnT[:, kt * CTXL : (kt + 1) * CTXL],
                                 start=(kt == 0), stop=(kt == 5))
            nc.vector.tensor_scalar_add(ck_sb[:], ps_ck[:, 0:CTXL], bctxk_sb[:])
            nc.vector.tensor_scalar_add(cv_sb[:, 0:CTXL], ps_cv[:, 0:CTXL],
                                        bctxv_sb[:])
            nc.vector.tensor_copy(cv_sb[:, CTXL : CTXL + 1], nullv_st[:])

            # ---- j-tile 18: [ctx 0:77 | null 77 | pad 78:128] ----
            VB = 18 * (DH + 1)
            nc.vector.memset(kT[:, 18 * 128 : JP], 0.0)
            nc.vector.tensor_copy(kT[:, 18 * 128 : 18 * 128 + CTXL], ck_sb[:])
            nc.vector.tensor_copy(kT[:, 18 * 128 + CTXL : 18 * 128 + CTXL + 1],
                                  nullk_st[:])
            nc.vector.memset(v_sb[:, VB : VB + DH + 1], 0.0)
            ps_cvt = pss.tile([128, 128], F32, tag="bT")
            nc.tensor.transpose(ps_cvt[0 : CTXL + 1, 0:64], cv_sb[:],
                                ident[:64, :64])
            nc.vector.tensor_copy(v_sb[0 : CTXL + 1, VB : VB + DH],
                                  ps_cvt[0 : CTXL + 1, 0:64])
            nc.vector.memset(v_sb[0 : CTXL + 1, VB + DH : VB + DH + 1], 1.0)

            # ---- kv projection (all tokens; LN folded; kT rs-scaled) ----
            KVC = 384
            for ch in range(N // KVC):
                sl = slice(ch * KVC, (ch + 1) * KVC)
                ps_k = pss.tile([128, KVC], F32, tag="bS")
                ps_v = pss.tile([128, KVC], F32, tag="bS")
                for kt in range(KT):
                    xs = x_bf[:, kt * N + ch * KVC : kt * N + (ch + 1) * KVC]
                    nc.tensor.matmul(ps_k[0:64, :],
                                     wkv_sb[:, kt * 2 * DH : kt * 2 * DH + DH],
                                     xs, start=(kt == 0), stop=False)
                    nc.tensor.matmul(ps_v[0:64, :],
                                     wkv_sb[:, kt * 2 * DH + DH : (kt + 1) * 2 * DH],
                                     xs, start=(kt == 0), stop=False)
                nc.tensor.matmul(ps_k[0:64, :], ncsk_sb[:], rows_bf[0:1, sl],
                                 start=False, stop=True)
                nc.tensor.matmul(ps_v[0:64, :], ncsv_sb[:], rows_bf[0:1, sl],
                                 start=False, stop=True)
                ps_bc = pss.tile([128, KVC], F32, tag="bS")
                nc.tensor.matmul(ps_bc[0:64, :], ones_blk[0:1, 0:64],
                                 rows[0:1, R_RS + ch * KVC : R_RS + (ch + 1) * KVC])
                kk = px2.tile([64, KVC], F32, tag="kk")
                nc.vector.tensor_copy(kk[:], ps_k[0:64, :])
                nc.vector.tensor_mul(kT[:, sl], kk[:], ps_bc[0:64, :])
                nc.vector.tensor_copy(vT[:, sl], ps_v[0:64, :])

            # ---- v tiles: transpose + rs scale + ones col ----
            for jt in range(18):
                ps_vt = pss.tile([128, 128], BF, tag="bT")
                nc.tensor.transpose(ps_vt[:, 0:64], vT[:, jt * 128 : (jt + 1) * 128],
                                    ident_bf[:64, :64])
                vb = jt * (DH + 1)
                nc.vector.tensor_scalar_mul(v_sb[:, vb : vb + DH], ps_vt[:, 0:64],
                                            stats[:, jt : jt + 1])
                nc.vector.memset(v_sb[:, vb + DH : vb + DH + 1], 1.0)

            # ---- q projection (own tokens; LN + 1/sqrt(dh) folded) ----
            for cc in range(NCH):
                sl = slice(cc * CH, (cc + 1) * CH)
                ps_rs = pss.tile([128, CH], F32, tag="bS")
                nc.tensor.matmul(ps_rs[:], ones_blk[0:1, :],
                                 rows[0:1, R_RS + cc * CH : R_RS + (cc + 1) * CH])
                rs_b = px2.tile([128, CH], F32, tag="rsb")
                nc.vector.tensor_copy(rs_b[:], ps_rs[:])
                for h in range(HEADS):
                    ps_q = pss.tile([128, CH], F32, tag="bS")
                    for kt in range(KT):
                        nc.tensor.matmul(
                            ps_q[0:64, :],
                            wq_sb[:, kt * C + h * DH : kt * C + (h + 1) * DH],
                            x_bf[:, kt * N + cc * CH : kt * N + (cc + 1) * CH],
                            start=(kt == 0), stop=False)
                    nc.tensor.matmul(ps_q[0:64, :],
                                     negcq_sb[0:1, h * DH : (h + 1) * DH],
                                     rows_bf[0:1, sl], start=False, stop=True)
                    nc.vector.tensor_mul(
                        qT[:, h * NH + cc * CH : h * NH + (cc + 1) * CH],
                        ps_q[0:64, :], rs_b[0:64, :])

        # ========= attention + output + LN2 + residual, per chunk =========
        with tc.tile_pool(name="attn", bufs=4) as pattn, \
             tc.tile_pool(name="outp", bufs=9) as pout, \
             tc.tile_pool(name="recp", bufs=4) as prec, \
             tc.tile_pool(name="p2p", bufs=2) as pp2, \
             tc.tile_pool(name="yp", bufs=3) as pyt, \
             tc.tile_pool(name="psatt", bufs=2, space="PSUM") as psA, \
             tc.tile_pool(name="psacc", bufs=2, space="PSUM") as psB, \
             tc.tile_pool(name="psmis", bufs=2, space="PSUM") as psM:
            for cc in range(NCH):
                ots = []
                for hg in range(HEADS // 2):
                    h0, h1 = 2 * hg, 2 * hg + 1
                    po0 = psB.tile([128, CH], F32, tag="po")
                    po1 = psB.tile([128, CH], F32, tag="po")
                    po = [po0, po1]
                    q0 = qT[:, h0 * NH + cc * CH : h0 * NH + (cc + 1) * CH]
                    q1 = qT[:, h1 * NH + cc * CH : h1 * NH + (cc + 1) * CH]
                    ats = [None] * JT
                    # software pipeline: attn.v lags one j-tile behind exp
                    for jt in range(JT):
                        ks = kT[:, jt * 128 : (jt + 1) * 128]
                        ps_s = psA.tile([128, 1024], F32, tag="sim")
                        nc.tensor.matmul(ps_s[:, 0:CH], ks, q0, start=True, stop=True)
                        nc.tensor.matmul(ps_s[:, 512 : 512 + CH], ks, q1,
                                         start=True, stop=True)
                        at = pattn.tile([128, 1024], BF, tag="at")
                        nc.scalar.activation(at[:], ps_s[:], AF.Exp)
                        ats[jt] = at
                        if jt > 0:
                            j0 = jt - 1
                            vs = v_sb[:, j0 * (DH + 1) : (j0 + 1) * (DH + 1)]
                            a0 = ats[j0]
                            nc.tensor.matmul(po[0][0:65, :], vs, a0[:, 0:CH],
                                             start=(j0 == 0), stop=False)
                            nc.tensor.matmul(po[1][0:65, :], vs,
                                             a0[:, 512 : 512 + CH],
                                             start=(j0 == 0), stop=False)
                            ats[j0] = None
                    j0 = JT - 1
                    vs = v_sb[:, j0 * (DH + 1) : (j0 + 1) * (DH + 1)]
                    nc.tensor.matmul(po[0][0:65, :], vs, ats[j0][:, 0:CH],
                                     start=False, stop=True)
                    nc.tensor.matmul(po[1][0:65, :], vs, ats[j0][:, 512 : 512 + CH],
                                     start=False, stop=True)
                    for i in range(2):
                        rec = prec.tile([65, CH], F32, tag="rec")
                        nc.vector.reciprocal(rec[64:65, :], po[i][64:65, :])
                        ps_rb = psM.tile([128, CH], F32, tag="mis")
                        nc.tensor.matmul(ps_rb[0:64, :], ones_blk[64:65, 0:64],
                                         rec[64:65, :])
                        rb_sb = prec.tile([64, CH], F32, tag="rbs")
                        nc.vector.tensor_copy(rb_sb[:], ps_rb[0:64, :])
                        ot = pout.tile([64, CH], BF, tag="ot")
                        nc.vector.tensor_mul(ot[:], po[i][0:64, :], rb_sb[:])
                        ots.append(ot)
                for ct in range(KT):
                    ps_p = psA.tile([128, 1024], F32, tag="sim")
                    for h in range(HEADS):
                        nc.tensor.matmul(
                            ps_p[:, 0:CH],
                            wout_sb[:, h * C + ct * 128 : h * C + (ct + 1) * 128],
                            ots[h][:], start=(h == 0), stop=(h == HEADS - 1))
                    nc.vector.tensor_copy(
                        projT[:, ct * NH + cc * CH : ct * NH + (cc + 1) * CH],
                        ps_p[:, 0:CH])

                # ---- LN2 stats for this chunk ----
                a, b = cc * CH, (cc + 1) * CH
                r2a, r2b = R_RS + cc * CH, R_RS + (cc + 1) * CH
                sca, scb = R_SC + cc * CH, R_SC + (cc + 1) * CH
                ps_m2 = psM.tile([128, CH], F32, tag="mis")
                for ct in range(KT):
                    nc.tensor.matmul(
                        ps_m2[0:1, :], ones_col[:],
                        projT[:, ct * NH + cc * CH : ct * NH + (cc + 1) * CH],
                        start=(ct == 0), stop=(ct == KT - 1))
                nc.scalar.mul(rows[0:1, a:b], ps_m2[0:1, :], 1.0 / C)
                p2 = pp2.tile([128, KT * CH], F32, tag="p2")
                ps_q2 = psM.tile([128, CH], F32, tag="mis")
                for ct in range(KT):
                    pslc = projT[:, ct * NH + cc * CH : ct * NH + (cc + 1) * CH]
                    nc.vector.tensor_mul(p2[:, ct * CH : (ct + 1) * CH], pslc, pslc)
                    nc.tensor.matmul(ps_q2[0:1, :], ones_col[:],
                                     p2[:, ct * CH : (ct + 1) * CH],
                                     start=(ct == 0), stop=(ct == KT - 1))
                nc.scalar.mul(rows[0:1, sca:scb], ps_q2[0:1, :], 1.0 / C)
                nc.vector.tensor_mul(rows[0:1, r2a:r2b],
                                     rows[0:1, a:b], rows[0:1, a:b])
                nc.vector.tensor_sub(rows[0:1, r2a:r2b],
                                     rows[0:1, sca:scb], rows[0:1, r2a:r2b])
                nc.scalar.activation(rows[0:1, r2a:r2b], rows[0:1, r2a:r2b],
                                     AF.Ln, bias=eps_col[0:1, :])
                nc.scalar.activation(rows[0:1, r2a:r2b], rows[0:1, r2a:r2b],
                                     AF.Exp, scale=-0.5)

                # ---- y = x + outg * (projT - mu2) * rs2 ----
                ps_bm = psM.tile([128, CH], F32, tag="mis")
                nc.tensor.matmul(ps_bm[:], ones_blk[0:1, :], rows[0:1, a:b])
                ps_br = psM.tile([128, CH], F32, tag="mis")
                nc.tensor.matmul(ps_br[:], ones_blk[0:1, :], rows[0:1, r2a:r2b])
                for ct in range(KT):
                    yt = pyt.tile([128, CH], F32, tag="yt")
                    pslice = projT[:, ct * NH + cc * CH : ct * NH + (cc + 1) * CH]
                    nc.vector.tensor_sub(yt[:], pslice, ps_bm[:])
                    nc.vector.tensor_mul(yt[:], yt[:], ps_br[:])
                    nc.vector.tensor_scalar_mul(yt[:], yt[:], outg_sb[:, ct : ct + 1])
                    nc.vector.tensor_add(
                        yt[:], yt[:],
                        x_sb[:, ct * N + cc * CH : ct * N + (cc + 1) * CH])
                    nc.sync.dma_start(
                        y[ct * 128 : (ct + 1) * 128, cc * CH : (cc + 1) * CH], yt[:])
    _split_multi_waits(nc)
    return nc


def _prep_inputs(x, context, norm_gamma, null_kv, Wq, Wkv, ctx_ln_g, ctx_ln_b,
                 Wctx, bctx, Wout, out_ln_g):
    import ml_dtypes
    bf = ml_dtypes.bfloat16
    f = np.float32
    x = np.asarray(x, f).reshape(4, C, N)
    context = np.asarray(context, f)
    g = np.asarray(norm_gamma, f)
    scale = 1.0 / np.sqrt(DH)
    wq_h = (g[:, None] * np.asarray(Wq, f)) * scale
    negcq_h = -wq_h.sum(0, dtype=np.float64).astype(f)[None, :]
    wkv_h = g[:, None] * np.asarray(Wkv, f)
    ncsk_h = -wkv_h[:, :DH].sum(0, dtype=np.float64).astype(f)[None, :]
    ncsv_h = -wkv_h[:, DH:].sum(0, dtype=np.float64).astype(f)[None, :]
    wctx_h = np.asarray(ctx_ln_g, f)[:, None] * np.asarray(Wctx, f)
    bctx_h = (np.asarray(bctx, f) + np.asarray(ctx_ln_b, f) @ np.asarray(Wctx, f))
    null = np.asarray(null_kv, f)
    wout_b = np.concatenate(
        [np.asarray(Wout, f)[h * DH:(h + 1) * DH, :] for h in range(HEADS)], axis=1)
    outg_h = np.ascontiguousarray(np.asarray(out_ln_g, f).reshape(KT, 128).T)

    shared = {
        "wq": np.ascontiguousarray(wq_h).astype(bf),
        "negcq": negcq_h.astype(bf),
        "wkv": np.ascontiguousarray(wkv_h).astype(bf),
        "ncsk": ncsk_h.astype(bf), "ncsv": ncsv_h.astype(bf),
        "wctx": np.ascontiguousarray(wctx_h),
        "bctxk": np.ascontiguousarray(bctx_h[:DH, None]),
        "bctxv": np.ascontiguousarray(bctx_h[DH:, None]),
        "nullkt": np.ascontiguousarray(null[0][:, None]),
        "nullv": np.ascontiguousarray(null[1][:, None]),
        "wout": np.ascontiguousarray(wout_b).astype(bf),
        "outg": outg_h,
    }
    in_maps = []
    for core in range(8):
        b, half = core // 2, core % 2
        m = dict(shared)
        m["x_own"] = np.ascontiguousarray(x[b][:, half * NH : (half + 1) * NH])
        m["x_oth"] = np.ascontiguousarray(x[b][:, (1 - half) * NH : (2 - half) * NH])
        m["ctxt"] = np.ascontiguousarray(context[b])
        in_maps.append(m)
    return in_maps


def kernel(**inputs):
    from concourse.bass_utils import run_bass_kernel_spmd

    if "nc" not in _cached:
        _cached["nc"] = _build_bass()
    nc = _cached["nc"]
    in_maps = _prep_inputs(**inputs)
    kw = {}
    if PROFILE:
        import importlib.util

        if "antenv.axon_hooks" not in sys.modules:
            spec = importlib.util.spec_from_file_location(
                "antenv.axon_hooks", "/opt/trn_rl_repo/antenv/axon_hooks.py")
            m = importlib.util.module_from_spec(spec)
            spec.loader.exec_module(m)
            sys.modules["antenv.axon_hooks"] = m
            import antenv

            antenv.axon_hooks = m
        kw = dict(trace=True, tmpdir=PROFILE_DIR)
    res = run_bass_kernel_spmd(nc, in_maps, list(range(8)), **kw)
    _cached["last"] = res
    out = np.empty((4, C, N), np.float32)
    for core in range(8):
        b, half = core // 2, core % 2
        out[b][:, half * NH : (half + 1) * NH] = res.results[core]["y"]
    return out.reshape(4, C, 48, 48)


# revision 17
# speedup vs baseline: 1.2241x; 1.2241x over previous
"""Trainium2 Bass kernel for nn_Attention_LR_65249143160949 (cross-attention block).

Sharding: 8 cores = 4 batches x 2 token-halves (1152 tokens each). Each core
computes k/v for its whole batch (cheap MQA single head, duplicated within the
pair) and q/attention/output for its own tokens. The host permutes tokens so
each core's own rows come first -> identical SPMD program, no collectives.

On-chip layout: features on partitions, tokens on the free axis (matches the
channels-first HBM layout; no input transpose). LayerNorm is folded into the
projections: q = rs_i * (x @ Wq' - mu_i * colsum(Wq')), with Wq' pre-scaled on
the host; k/v analogous (rank-1 -colsum*mu matmul accumulated into the same
psum group). Attention runs in sim^T layout (keys on partitions, query tokens
on the free axis): kT is pre-scaled by rs_j so softmax is a plain exp; the
denominator comes free as a ones-column appended to v (row 64 of the out
psum); out^T columns are normalized by a PE-broadcast reciprocal row.
Per-token LN stats come from PE matmuls (ones as one operand), never from
cross-partition vector ops.

Precision: fp32 end-to-end math except the five big matmul groups
(q/kv/sim/attn.v/out-proj), whose operands are bf16 with fp32 PSUM
accumulation. LN statistics, softmax normalization, and the residual path
stay fp32.

Two walrus quirks are handled: every TPB instruction holds at most ONE sync
wait (extras are split onto same-engine NoOps by _split_multi_waits), and
custom DVE ops are unavailable (exact reciprocal is used).
"""

import sys

import numpy as np

if "/opt/trn_rl_repo" not in sys.path:
    sys.path.insert(0, "/opt/trn_rl_repo")

C = 512          # channels
N = 2304         # tokens per batch (48*48)
NH = 1152        # tokens per core
HEADS = 8
DH = 64
CTXL = 77
CTXD = 768
JT = 19          # j tiles of 128: 18 img + 1 (ctx 0:77 | null 77 | pad)
JP = JT * 128
CHUNKS = [(0, 512), (512, 512), (1024, 128)]  # (start, len) token chunks
NCH = len(CHUNKS)
KT = 4           # C / 128
EPS = 1e-5

PROFILE = False
PROFILE_DIR = None

_cached = {}


def _split_multi_waits(nc):
    """Walrus codegen supports one sync-wait per TPB instruction (the EVENTS
    struct has a single wait slot). Tile attaches several. Split the extras
    onto same-engine NoOps inserted just before each instruction."""
    import concourse.mybir as mybir

    n = 0
    for fn in nc.m.functions:
        for bb in fn.blocks:
            insts = bb.instructions
            i = 0
            while i < len(insts):
                ins = insts[i]
                si = getattr(ins, "sync_info", None)
                if si is not None and si.on_wait and len(si.on_wait) > 1:
                    waits = list(si.on_wait)
                    for w in waits[:-1]:
                        n += 1
                        nop = mybir.InstNoOp(name=f"WSPLIT-{n}", engine=ins.engine)
                        nop.sync_info = mybir.SyncInfo(on_wait=[w], on_update=[])
                        insts.insert(i, nop)
                        i += 1
                    ins.sync_info = mybir.SyncInfo(
                        on_wait=[waits[-1]], on_update=si.on_update)
                i += 1
    return n


def _build_bass():
    import concourse.bass as bass
    import concourse.mybir as mybir
    import concourse.tile as tile
    from concourse.masks import make_identity
    from contextlib import ExitStack

    F32 = mybir.dt.float32
    BF = mybir.dt.bfloat16
    AF = mybir.ActivationFunctionType
    ALU = mybir.AluOpType

    nc = bass.Bass()
    x_own = nc.declare_dram_parameter("x_own", [C, NH], F32, isOutput=False)
    x_oth = nc.declare_dram_parameter("x_oth", [C, NH], F32, isOutput=False)
    ctxt = nc.declare_dram_parameter("ctxt", [CTXL, CTXD], F32, isOutput=False)
    wq = nc.declare_dram_parameter("wq", [C, C], BF, isOutput=False)
    negcq = nc.declare_dram_parameter("negcq", [1, C], BF, isOutput=False)
    wkv = nc.declare_dram_parameter("wkv", [C, 2 * DH], BF, isOutput=False)
    ncsk = nc.declare_dram_parameter("ncsk", [1, DH], BF, isOutput=False)
    ncsv = nc.declare_dram_parameter("ncsv", [1, DH], BF, isOutput=False)
    wctx = nc.declare_dram_parameter("wctx", [CTXD, 2 * DH], F32, isOutput=False)
    bctxk = nc.declare_dram_parameter("bctxk", [DH, 1], F32, isOutput=False)
    bctxv = nc.declare_dram_parameter("bctxv", [DH, 1], F32, isOutput=False)
    nullkt = nc.declare_dram_parameter("nullkt", [DH, 1], F32, isOutput=False)
    nullv = nc.declare_dram_parameter("nullv", [DH, 1], F32, isOutput=False)
    wout = nc.declare_dram_parameter("wout", [DH, HEADS * C], BF, isOutput=False)
    outg = nc.declare_dram_parameter("outg", [128, KT], F32, isOutput=False)
    y = nc.declare_dram_parameter("y", [C, NH], F32, isOutput=True)

    with tile.TileContext(nc) as tc, ExitStack() as ctx:
        pconst = ctx.enter_context(tc.tile_pool(name="const", bufs=1))
        pbig = ctx.enter_context(tc.tile_pool(name="big", bufs=1))

        ident = pconst.tile([128, 128], F32)
        make_identity(nc, ident[:])
        ident_bf = pconst.tile([128, 128], BF)
        make_identity(nc, ident_bf[:])
        ones_col = pconst.tile([128, 1], F32)
        nc.vector.memset(ones_col[:], 1.0)
        ones_blk = pconst.tile([128, 128], F32)
        nc.vector.memset(ones_blk[:], 1.0)
        eps_col = pconst.tile([128, 1], F32)
        nc.vector.memset(eps_col[:], EPS)

        x_sb = pbig.tile([128, KT * N], F32)         # kt-major; own rows first
        x_bf = pbig.tile([128, KT * N], BF)
        qT = pbig.tile([128, (HEADS // 2) * NH], BF)  # head-pair blocks
        kT2 = pbig.tile([128, JP], BF)               # rs-scaled keys, both halves
        v_sb = pbig.tile([128, JT * (DH + 1)], BF)   # per j-tile [128, 64+ones]
        projT = pbig.tile([128, KT * NH], F32)
        stats = pbig.tile([128, 40], F32)            # col jt: rs_j (v scaling)
        wout_sb = pbig.tile([64, HEADS * C], BF)
        outg_sb = pbig.tile([128, KT], F32)
        # per-token stat rows on partition 0: mu 0:N | rs N:2N
        # (LN2 reuses per cc: mu2 at cc*CH, rs2 at N+cc*CH, ex2 at 2N+cc*CH)
        rows = pbig.tile([1, 2 * N + NH], F32)
        rows_bf = pbig.tile([1, N], BF)
        R_RS, R_SC = N, 2 * N

        nc.sync.dma_start(wout_sb[:], wout[:, :])
        nc.sync.dma_start(outg_sb[:], outg[:, :])

        with tc.tile_pool(name="load", bufs=1) as pload, \
             tc.tile_pool(name="x2p", bufs=2) as px2, \
             tc.tile_pool(name="pss", bufs=2, space="PSUM") as pss:
            # setup psum tags: b1 [<=64,384]x2, bS [128,<=512]x4, bT [128,128]x2
            wq_sb = pload.tile([128, KT * C], BF)
            wkv_sb = pload.tile([128, KT * 2 * DH], BF)
            wctx_sb = pload.tile([128, CTXD], F32)
            negcq_sb = pload.tile([1, C], BF)
            ncsk_sb = pload.tile([1, DH], BF)
            ncsv_sb = pload.tile([1, DH], BF)
            bctxk_sb = pload.tile([DH, 1], F32)
            bctxv_sb = pload.tile([DH, 1], F32)
            vT = pload.tile([64, N], BF)
            ck_sb = pload.tile([64, CTXL], F32)
            cv_sb = pload.tile([64, CTXL + 1], F32)
            nullk_st = pload.tile([DH, 1], F32)
            nullv_st = pload.tile([DH, 1], F32)
            ctx_sb = pload.tile([CTXL, CTXD], F32)
            ctxnT = pload.tile([128, 6 * CTXL], F32)
            ex2 = pload.tile([1, N], F32)

            x_v = x_sb[:].rearrange("p (k n) -> p k n", k=KT)
            nc.sync.dma_start(x_v[:, :, 0:NH],
                              x_own[:].rearrange("(k p) n -> p k n", p=128))
            nc.sync.dma_start(x_v[:, :, NH:N],
                              x_oth[:].rearrange("(k p) n -> p k n", p=128))
            nc.sync.dma_start(wq_sb[:].rearrange("p (k n) -> p k n", k=KT),
                              wq[:].rearrange("(k p) n -> p k n", p=128))
            nc.sync.dma_start(wkv_sb[:].rearrange("p (k n) -> p k n", k=KT),
                              wkv[:].rearrange("(k p) n -> p k n", p=128))
            nc.sync.dma_start(wctx_sb[:].rearrange("p (k n) -> p k n", k=6),
                              wctx[:].rearrange("(k p) n -> p k n", p=128))
            nc.sync.dma_start(negcq_sb[:], negcq[:, :])
            nc.sync.dma_start(ncsk_sb[:], ncsk[:, :])
            nc.sync.dma_start(ncsv_sb[:], ncsv[:, :])
            nc.sync.dma_start(bctxk_sb[:], bctxk[:, :])
            nc.sync.dma_start(bctxv_sb[:], bctxv[:, :])
            nc.sync.dma_start(ctx_sb[:], ctxt[:, :])
            nc.sync.dma_start(nullk_st[:], nullkt[:, :])
            nc.sync.dma_start(nullv_st[:], nullv[:, :])

            for kt in range(KT):
                nc.vector.tensor_copy(x_bf[:, kt * N : (kt + 1) * N],
                                      x_sb[:, kt * N : (kt + 1) * N])

            # ---- LN1 stats (row form): mu, then rs = exp(-0.5 ln(var+eps)) ----
            for ch in range(6):
                sl = slice(ch * 384, (ch + 1) * 384)
                ps_r1 = pss.tile([64, 384], F32, tag="b1")
                for kt in range(KT):
                    nc.tensor.matmul(
                        ps_r1[0:1, :], ones_col[:],
                        x_sb[:, kt * N + ch * 384 : kt * N + (ch + 1) * 384],
                        start=(kt == 0), stop=(kt == KT - 1))
                nc.scalar.mul(rows[0:1, sl], ps_r1[0:1, :], 1.0 / C)
                nc.vector.tensor_copy(rows_bf[0:1, sl], rows[0:1, sl])
            for ch in range(6):
                x2 = px2.tile([128, KT * 384], F32, tag="x2")
                ps_r2 = pss.tile([64, 384], F32, tag="b1")
                for kt in range(KT):
                    xs = x_sb[:, kt * N + ch * 384 : kt * N + (ch + 1) * 384]
                    nc.vector.tensor_mul(x2[:, kt * 384 : (kt + 1) * 384], xs, xs)
                    nc.tensor.matmul(
                        ps_r2[0:1, :], ones_col[:], x2[:, kt * 384 : (kt + 1) * 384],
                        start=(kt == 0), stop=(kt == KT - 1))
                nc.scalar.mul(ex2[0:1, ch * 384 : (ch + 1) * 384],
                              ps_r2[0:1, :], 1.0 / C)
            for ch in range(6):
                a, b = R_RS + ch * 384, R_RS + (ch + 1) * 384
                mu = rows[0:1, ch * 384 : (ch + 1) * 384]
                nc.vector.tensor_mul(rows[0:1, a:b], mu, mu)
                nc.vector.tensor_sub(rows[0:1, a:b],
                                     ex2[0:1, ch * 384 : (ch + 1) * 384],
                                     rows[0:1, a:b])
                nc.scalar.activation(rows[0:1, a:b], rows[0:1, a:b], AF.Ln,
                                     bias=eps_col[0:1, :])
                nc.scalar.activation(rows[0:1, a:b], rows[0:1, a:b], AF.Exp,
                                     scale=-0.5)
            # rs as per-partition columns (v scaling)
            for jt in range(18):
                ps_c = pss.tile([128, 128], F32, tag="bT")
                nc.tensor.matmul(ps_c[:, 0:1],
                                 rows[0:1, R_RS + jt * 128 : R_RS + (jt + 1) * 128],
                                 ones_col[0:1, :])
                nc.vector.tensor_copy(stats[:, jt : jt + 1], ps_c[:, 0:1])

            # ---- context: LN (layout A, bn_stats) + k/v projection ----
            cstat = pload.tile([CTXL, 3, 6], F32)
            for sg in range(3):
                nc.vector.bn_stats(cstat[:, sg, :],
                                   ctx_sb[:, sg * 256 : (sg + 1) * 256])
            cmv = pload.tile([CTXL, 2], F32)
            nc.vector.bn_aggr(cmv[:], cstat[:])
            nc.scalar.activation(cmv[:, 1:2], cmv[:, 1:2], AF.Ln,
                                 bias=eps_col[0:CTXL, :])
            nc.scalar.activation(cmv[:, 1:2], cmv[:, 1:2], AF.Exp, scale=-0.5)
            nc.vector.tensor_scalar(
                out=ctx_sb[:], in0=ctx_sb[:],
                scalar1=cmv[:, 0:1], scalar2=cmv[:, 1:2],
                op0=ALU.subtract, op1=ALU.mult)
            for kt in range(6):
                ps_ct = pss.tile([128, 128], F32, tag="bT")
                nc.tensor.transpose(ps_ct[:, 0:CTXL],
                                    ctx_sb[:, kt * 128 : (kt + 1) * 128],
                                    ident[:CTXL, :CTXL])
                nc.vector.tensor_copy(ctxnT[:, kt * CTXL : (kt + 1) * CTXL],
                                      ps_ct[:, 0:CTXL])
            ps_ck = pss.tile([64, 384], F32, tag="b1")
            ps_cv = pss.tile([64, 384], F32, tag="b1")
            for kt in range(6):
                nc.tensor.matmul(ps_ck[:, 0:CTXL],
                                 wctx_sb[:, kt * 128 : kt * 128 + DH],
                                 ctxnT[:, kt * CTXL : (kt + 1) * CTXL],
                                 start=(kt == 0), stop=(kt == 5))
                nc.tensor.matmul(ps_cv[:, 0:CTXL],
                                 wctx_sb[:, kt * 128 + DH : (kt + 1) * 128],
                                 ctxnT[:, kt * CTXL : (kt + 1) * CTXL],
                                 start=(kt == 0), stop=(kt == 5))
            nc.vector.tensor_scalar_add(ck_sb[:], ps_ck[:, 0:CTXL], bctxk_sb[:])
            nc.vector.tensor_scalar_add(cv_sb[:, 0:CTXL], ps_cv[:, 0:CTXL],
                                        bctxv_sb[:])
            nc.vector.tensor_copy(cv_sb[:, CTXL : CTXL + 1], nullv_st[:])

            # ---- j-tile 18: [ctx 0:77 | null 77 | pad 78:128] ----
            VB = 18 * (DH + 1)
            nc.vector.memset(kT2[0:64, 18 * 128 : JP], 0.0)
            nc.vector.tensor_copy(kT2[0:64, 18 * 128 : 18 * 128 + CTXL], ck_sb[:])
            nc.vector.tensor_copy(kT2[0:64, 18 * 128 + CTXL : 18 * 128 + CTXL + 1],
                                  nullk_st[:])
            nc.vector.memset(v_sb[:, VB : VB + DH + 1], 0.0)
            ps_cvt = pss.tile([128, 128], F32, tag="bT")
            nc.tensor.transpose(ps_cvt[0 : CTXL + 1, 0:64], cv_sb[:],
                                ident[:64, :64])
            nc.vector.tensor_copy(v_sb[0 : CTXL + 1, VB : VB + DH],
                                  ps_cvt[0 : CTXL + 1, 0:64])
            nc.vector.memset(v_sb[0 : CTXL + 1, VB + DH : VB + DH + 1], 1.0)

            # ---- kv projection (all tokens; LN folded; kT rs-scaled) ----
            KVC = 384
            for ch in range(N // KVC):
                sl = slice(ch * KVC, (ch + 1) * KVC)
                ps_k = pss.tile([128, KVC], F32, tag="bS")
                ps_v = pss.tile([128, KVC], F32, tag="bS")
                for kt in range(KT):
                    xs = x_bf[:, kt * N + ch * KVC : kt * N + (ch + 1) * KVC]
                    nc.tensor.matmul(ps_k[0:64, :],
                                     wkv_sb[:, kt * 2 * DH : kt * 2 * DH + DH],
                                     xs, start=(kt == 0), stop=False)
                    nc.tensor.matmul(ps_v[0:64, :],
                                     wkv_sb[:, kt * 2 * DH + DH : (kt + 1) * 2 * DH],
                                     xs, start=(kt == 0), stop=False)
                nc.tensor.matmul(ps_k[0:64, :], ncsk_sb[:], rows_bf[0:1, sl],
                                 start=False, stop=True)
                nc.tensor.matmul(ps_v[0:64, :], ncsv_sb[:], rows_bf[0:1, sl],
                                 start=False, stop=True)
                ps_bc = pss.tile([128, KVC], F32, tag="bS")
                nc.tensor.matmul(ps_bc[0:64, :], ones_blk[0:1, 0:64],
                                 rows[0:1, R_RS + ch * KVC : R_RS + (ch + 1) * KVC])
                kk = px2.tile([64, KVC], F32, tag="kk")
                nc.vector.tensor_copy(kk[:], ps_k[0:64, :])
                nc.vector.tensor_mul(kT2[0:64, sl], kk[:], ps_bc[0:64, :])
                nc.vector.tensor_copy(vT[:, sl], ps_v[0:64, :])

            # ---- v tiles: transpose + rs scale + ones col ----
            for jt in range(18):
                ps_vt = pss.tile([128, 128], BF, tag="bT")
                nc.tensor.transpose(ps_vt[:, 0:64], vT[:, jt * 128 : (jt + 1) * 128],
                                    ident_bf[:64, :64])
                vb = jt * (DH + 1)
                nc.vector.tensor_scalar_mul(v_sb[:, vb : vb + DH], ps_vt[:, 0:64],
                                            stats[:, jt : jt + 1])
                nc.vector.memset(v_sb[:, vb + DH : vb + DH + 1], 1.0)

            # ---- duplicate kT to partitions 64:128 (sbuf->sbuf DMA) ----
            nc.sync.dma_start(kT2[64:128, :], kT2[0:64, :])

            # ---- q projection (head pairs; LN + 1/sqrt(dh) folded) ----
            for a0, ln in CHUNKS:
                sl = slice(a0, a0 + ln)
                ps_rs = pss.tile([128, 512], F32, tag="bS")
                nc.tensor.matmul(ps_rs[:, 0:ln], ones_blk[0:1, :],
                                 rows[0:1, R_RS + a0 : R_RS + a0 + ln])
                rs_b = px2.tile([128, 512], F32, tag="rsb")
                nc.vector.tensor_copy(rs_b[:, 0:ln], ps_rs[:, 0:ln])
                for hg in range(HEADS // 2):
                    ps_q = pss.tile([128, 512], F32, tag="bS")
                    for kt in range(KT):
                        nc.tensor.matmul(
                            ps_q[:, 0:ln],
                            wq_sb[:, kt * C + hg * 128 : kt * C + (hg + 1) * 128],
                            x_bf[:, kt * N + a0 : kt * N + a0 + ln],
                            start=(kt == 0), stop=False)
                    nc.tensor.matmul(ps_q[:, 0:ln],
                                     negcq_sb[0:1, hg * 128 : (hg + 1) * 128],
                                     rows_bf[0:1, sl], start=False, stop=True)
                    nc.vector.tensor_mul(
                        qT[:, hg * NH + a0 : hg * NH + a0 + ln],
                        ps_q[:, 0:ln], rs_b[:, 0:ln])

        # ========= attention + output + LN2 + residual, per chunk =========
        with tc.tile_pool(name="attn", bufs=4) as pattn, \
             tc.tile_pool(name="outp", bufs=9) as pout, \
             tc.tile_pool(name="recp", bufs=4) as prec, \
             tc.tile_pool(name="p2p", bufs=2) as pp2, \
             tc.tile_pool(name="yp", bufs=3) as pyt, \
             tc.tile_pool(name="psatt", bufs=2, space="PSUM") as psA, \
             tc.tile_pool(name="psacc", bufs=2, space="PSUM") as psB, \
             tc.tile_pool(name="psmis", bufs=2, space="PSUM") as psM:
            for cc, (a0, ln) in enumerate(CHUNKS):
                ots = []
                for hg in range(HEADS // 2):
                    po0 = psB.tile([128, 512], F32, tag="po")
                    po1 = psB.tile([128, 512], F32, tag="po")
                    po = [po0, po1]
                    q0 = qT[0:64, hg * NH + a0 : hg * NH + a0 + ln]
                    q1 = qT[64:128, hg * NH + a0 : hg * NH + a0 + ln]
                    ats = [None] * JT
                    # software pipeline: attn.v lags one j-tile behind exp;
                    # the two sims of a j-tile run row-packed (concurrent)
                    for jt in range(JT):
                        ps_s = psA.tile([128, 1024], F32, tag="sim")
                        nc.tensor.matmul(ps_s[:, 0:ln],
                                         kT2[0:64, jt * 128 : (jt + 1) * 128],
                                         q0, start=True, stop=True)
                        nc.tensor.matmul(ps_s[:, 512 : 512 + ln],
                                         kT2[64:128, jt * 128 : (jt + 1) * 128],
                                         q1, start=True, stop=True)
                        at = pattn.tile([128, 1024], BF, tag="at")
                        if ln == 512:
                            nc.scalar.activation(at[:], ps_s[:], AF.Exp)
                        else:
                            nc.scalar.activation(at[:, 0:ln], ps_s[:, 0:ln], AF.Exp)
                            nc.scalar.activation(at[:, 512 : 512 + ln],
                                                 ps_s[:, 512 : 512 + ln], AF.Exp)
                        ats[jt] = at
                        if jt > 0:
                            j0 = jt - 1
                            vs = v_sb[:, j0 * (DH + 1) : (j0 + 1) * (DH + 1)]
                            nc.tensor.matmul(po[0][0:65, 0:ln], vs,
                                             ats[j0][:, 0:ln],
                                             start=(j0 == 0), stop=False)
                            nc.tensor.matmul(po[1][0:65, 0:ln], vs,
                                             ats[j0][:, 512 : 512 + ln],
                                             start=(j0 == 0), stop=False)
                            ats[j0] = None
                    j0 = JT - 1
                    vs = v_sb[:, j0 * (DH + 1) : (j0 + 1) * (DH + 1)]
                    nc.tensor.matmul(po[0][0:65, 0:ln], vs, ats[j0][:, 0:ln],
                                     start=False, stop=True)
                    nc.tensor.matmul(po[1][0:65, 0:ln], vs,
                                     ats[j0][:, 512 : 512 + ln],
                                     start=False, stop=True)
                    for i in range(2):
                        rec = prec.tile([65, 512], F32, tag="rec")
                        nc.vector.reciprocal(rec[64:65, 0:ln], po[i][64:65, 0:ln])
                        ps_rb = psM.tile([128, 512], F32, tag="mis")
                        nc.tensor.matmul(ps_rb[0:64, 0:ln], ones_blk[64:65, 0:64],
                                         rec[64:65, 0:ln])
                        rb_sb = prec.tile([64, 512], F32, tag="rbs")
                        nc.vector.tensor_copy(rb_sb[:, 0:ln], ps_rb[0:64, 0:ln])
                        ot = pout.tile([64, 512], BF, tag="ot")
                        nc.vector.tensor_mul(ot[:, 0:ln], po[i][0:64, 0:ln],
                                             rb_sb[:, 0:ln])
                        ots.append(ot)
                for ct in range(KT):
                    ps_p = psA.tile([128, 1024], F32, tag="sim")
                    for h in range(HEADS):
                        nc.tensor.matmul(
                            ps_p[:, 0:ln],
                            wout_sb[:, h * C + ct * 128 : h * C + (ct + 1) * 128],
                            ots[h][:, 0:ln], start=(h == 0), stop=(h == HEADS - 1))
                    nc.vector.tensor_copy(
                        projT[:, ct * NH + a0 : ct * NH + a0 + ln],
                        ps_p[:, 0:ln])

                # ---- LN2 stats for this chunk ----
                ra, rb2 = R_RS + a0, R_RS + a0 + ln
                sca, scb = R_SC + a0, R_SC + a0 + ln
                ps_m2 = psM.tile([128, 512], F32, tag="mis")
                for ct in range(KT):
                    nc.tensor.matmul(
                        ps_m2[0:1, 0:ln], ones_col[:],
                        projT[:, ct * NH + a0 : ct * NH + a0 + ln],
                        start=(ct == 0), stop=(ct == KT - 1))
                nc.scalar.mul(rows[0:1, a0 : a0 + ln], ps_m2[0:1, 0:ln], 1.0 / C)
                p2 = pp2.tile([128, KT * 512], F32, tag="p2")
                ps_q2 = psM.tile([128, 512], F32, tag="mis")
                for ct in range(KT):
                    pslc = projT[:, ct * NH + a0 : ct * NH + a0 + ln]
                    nc.vector.tensor_mul(p2[:, ct * 512 : ct * 512 + ln], pslc, pslc)
                    nc.tensor.matmul(ps_q2[0:1, 0:ln], ones_col[:],
                                     p2[:, ct * 512 : ct * 512 + ln],
                                     start=(ct == 0), stop=(ct == KT - 1))
                nc.scalar.mul(rows[0:1, sca:scb], ps_q2[0:1, 0:ln], 1.0 / C)
                nc.vector.tensor_mul(rows[0:1, ra:rb2],
                                     rows[0:1, a0 : a0 + ln], rows[0:1, a0 : a0 + ln])
                nc.vector.tensor_sub(rows[0:1, ra:rb2],
                                     rows[0:1, sca:scb], rows[0:1, ra:rb2])
                nc.scalar.activation(rows[0:1, ra:rb2], rows[0:1, ra:rb2],
                                     AF.Ln, bias=eps_col[0:1, :])
                nc.scalar.activation(rows[0:1, ra:rb2], rows[0:1, ra:rb2],
                                     AF.Exp, scale=-0.5)

                # ---- y = x + outg * (projT - mu2) * rs2 ----
                ps_bm = psM.tile([128, 512], F32, tag="mis")
                nc.tensor.matmul(ps_bm[:, 0:ln], ones_blk[0:1, :],
                                 rows[0:1, a0 : a0 + ln])
                ps_br = psM.tile([128, 512], F32, tag="mis")
                nc.tensor.matmul(ps_br[:, 0:ln], ones_blk[0:1, :], rows[0:1, ra:rb2])
                for ct in range(KT):
                    yt = pyt.tile([128, 512], F32, tag="yt")
                    pslice = projT[:, ct * NH + a0 : ct * NH + a0 + ln]
                    nc.vector.tensor_sub(yt[:, 0:ln], pslice, ps_bm[:, 0:ln])
                    nc.vector.tensor_mul(yt[:, 0:ln], yt[:, 0:ln], ps_br[:, 0:ln])
                    nc.vector.tensor_scalar_mul(yt[:, 0:ln], yt[:, 0:ln],
                                                outg_sb[:, ct : ct + 1])
                    nc.vector.tensor_add(
                        yt[:, 0:ln], yt[:, 0:ln],
                        x_sb[:, ct * N + a0 : ct * N + a0 + ln])
                    nc.sync.dma_start(
                        y[ct * 128 : (ct + 1) * 128, a0 : a0 + ln], yt[:, 0:ln])
    _split_multi_waits(nc)
    return nc


def _prep_inputs(x, context, norm_gamma, null_kv, Wq, Wkv, ctx_ln_g, ctx_ln_b,
                 Wctx, bctx, Wout, out_ln_g):
    import ml_dtypes
    bf = ml_dtypes.bfloat16
    f = np.float32
    x = np.asarray(x, f).reshape(4, C, N)
    context = np.asarray(context, f)
    g = np.asarray(norm_gamma, f)
    scale = 1.0 / np.sqrt(DH)
    wq_h = (g[:, None] * np.asarray(Wq, f)) * scale
    negcq_h = -wq_h.sum(0, dtype=np.float64).astype(f)[None, :]
    wkv_h = g[:, None] * np.asarray(Wkv, f)
    ncsk_h = -wkv_h[:, :DH].sum(0, dtype=np.float64).astype(f)[None, :]
    ncsv_h = -wkv_h[:, DH:].sum(0, dtype=np.float64).astype(f)[None, :]
    wctx_h = np.asarray(ctx_ln_g, f)[:, None] * np.asarray(Wctx, f)
    bctx_h = (np.asarray(bctx, f) + np.asarray(ctx_ln_b, f) @ np.asarray(Wctx, f))
    null = np.asarray(null_kv, f)
    wout_b = np.concatenate(
        [np.asarray(Wout, f)[h * DH:(h + 1) * DH, :] for h in range(HEADS)], axis=1)
    outg_h = np.ascontiguousarray(np.asarray(out_ln_g, f).reshape(KT, 128).T)

    shared = {
        "wq": np.ascontiguousarray(wq_h).astype(bf),
        "negcq": negcq_h.astype(bf),
        "wkv": np.ascontiguousarray(wkv_h).astype(bf),
        "ncsk": ncsk_h.astype(bf), "ncsv": ncsv_h.astype(bf),
        "wctx": np.ascontiguousarray(wctx_h),
        "bctxk": np.ascontiguousarray(bctx_h[:DH, None]),
        "bctxv": np.ascontiguousarray(bctx_h[DH:, None]),
        "nullkt": np.ascontiguousarray(null[0][:, None]),
        "nullv": np.ascontiguousarray(null[1][:, None]),
        "wout": np.ascontiguousarray(wout_b).astype(bf),
        "outg": outg_h,
    }
    in_maps = []
    for core in range(8):
        b, half = core // 2, core % 2
        m = dict(shared)
        m["x_own"] = np.ascontiguousarray(x[b][:, half * NH : (half + 1) * NH])
        m["x_oth"] = np.ascontiguousarray(x[b][:, (1 - half) * NH : (2 - half) * NH])
        m["ctxt"] = np.ascontiguousarray(context[b])
        in_maps.append(m)
    return in_maps


def kernel(**inputs):
    from concourse.bass_utils import run_bass_kernel_spmd

    if "nc" not in _cached:
        _cached["nc"] = _build_bass()
    nc = _cached["nc"]
    in_maps = _prep_inputs(**inputs)
    kw = {}
    if PROFILE:
        import importlib.util

        if "antenv.axon_hooks" not in sys.modules:
            spec = importlib.util.spec_from_file_location(
                "antenv.axon_hooks", "/opt/trn_rl_repo/antenv/axon_hooks.py")
            m = importlib.util.module_from_spec(spec)
            spec.loader.exec_module(m)
            sys.modules["antenv.axon_hooks"] = m
            import antenv

            antenv.axon_hooks = m
        kw = dict(trace=True, tmpdir=PROFILE_DIR)
    res = run_bass_kernel_spmd(nc, in_maps, list(range(8)), **kw)
    _cached["last"] = res
    out = np.empty((4, C, N), np.float32)
    for core in range(8):
        b, half = core // 2, core % 2
        out[b][:, half * NH : (half + 1) * NH] = res.results[core]["y"]
    return out.reshape(4, C, 48, 48)


# revision 19
# speedup vs baseline: 1.2271x; 1.0024x over previous
"""Trainium2 Bass kernel for nn_Attention_LR_65249143160949 (cross-attention block).

Sharding: 8 cores = 4 batches x 2 token-halves (1152 tokens each). Each core
computes k/v for its whole batch (cheap MQA single head, duplicated within the
pair) and q/attention/output for its own tokens. The host permutes tokens so
each core's own rows come first -> identical SPMD program, no collectives.

On-chip layout: features on partitions, tokens on the free axis (matches the
channels-first HBM layout; no input transpose). LayerNorm is folded into the
projections: q = rs_i * (x @ Wq' - mu_i * colsum(Wq')), with Wq' pre-scaled on
the host; k/v analogous (rank-1 -colsum*mu matmul accumulated into the same
psum group). Attention runs in sim^T layout (keys on partitions, query tokens
on the free axis): kT is pre-scaled by rs_j so softmax is a plain exp; the
denominator comes free as a ones-column appended to v (row 64 of the out
psum); out^T columns are normalized by a PE-broadcast reciprocal row.
Per-token LN stats come from PE matmuls (ones as one operand), never from
cross-partition vector ops.

Precision: fp32 end-to-end math except the five big matmul groups
(q/kv/sim/attn.v/out-proj), whose operands are bf16 with fp32 PSUM
accumulation. LN statistics, softmax normalization, and the residual path
stay fp32.

Two walrus quirks are handled: every TPB instruction holds at most ONE sync
wait (extras are split onto same-engine NoOps by _split_multi_waits), and
custom DVE ops are unavailable (exact reciprocal is used).
"""

import sys

import numpy as np

if "/opt/trn_rl_repo" not in sys.path:
    sys.path.insert(0, "/opt/trn_rl_repo")

C = 512          # channels
N = 2304         # tokens per batch (48*48)
NH = 1152        # tokens per core
HEADS = 8
DH = 64
CTXL = 77
CTXD = 768
JT = 19          # j tiles of 128: 18 img + 1 (ctx 0:77 | null 77 | pad)
JP = JT * 128
CHUNKS = [(0, 512), (512, 512), (1024, 128)]  # (start, len) token chunks
NCH = len(CHUNKS)
KT = 4           # C / 128
EPS = 1e-5

PROFILE = False
PROFILE_DIR = None

_cached = {}


def _split_multi_waits(nc):
    """Walrus codegen supports one sync-wait per TPB instruction (the EVENTS
    struct has a single wait slot). Tile attaches several. Split the extras
    onto same-engine NoOps inserted just before each instruction."""
    import concourse.mybir as mybir

    n = 0
    for fn in nc.m.functions:
        for bb in fn.blocks:
            insts = bb.instructions
            i = 0
            while i < len(insts):
                ins = insts[i]
                si = getattr(ins, "sync_info", None)
                if si is not None and si.on_wait and len(si.on_wait) > 1:
                    waits = list(si.on_wait)
                    for w in waits[:-1]:
                        n += 1
                        nop = mybir.InstNoOp(name=f"WSPLIT-{n}", engine=ins.engine)
                        nop.sync_info = mybir.SyncInfo(on_wait=[w], on_update=[])
                        insts.insert(i, nop)
                        i += 1
                    ins.sync_info = mybir.SyncInfo(
                        on_wait=[waits[-1]], on_update=si.on_update)
                i += 1
    return n


def _build_bass():
    import concourse.bass as bass
    import concourse.mybir as mybir
    import concourse.tile as tile
    from concourse.masks import make_identity
    from contextlib import ExitStack

    F32 = mybir.dt.float32
    BF = mybir.dt.bfloat16
    AF = mybir.ActivationFunctionType
    ALU = mybir.AluOpType

    nc = bass.Bass()
    x_own = nc.declare_dram_parameter("x_own", [C, NH], F32, isOutput=False)
    x_oth = nc.declare_dram_parameter("x_oth", [C, NH], F32, isOutput=False)
    ctxt = nc.declare_dram_parameter("ctxt", [CTXL, CTXD], F32, isOutput=False)
    wq = nc.declare_dram_parameter("wq", [C, C], BF, isOutput=False)
    negcq = nc.declare_dram_parameter("negcq", [1, C], BF, isOutput=False)
    wkv = nc.declare_dram_parameter("wkv", [C, 2 * DH], BF, isOutput=False)
    ncsk = nc.declare_dram_parameter("ncsk", [1, DH], BF, isOutput=False)
    ncsv = nc.declare_dram_parameter("ncsv", [1, DH], BF, isOutput=False)
    wctx = nc.declare_dram_parameter("wctx", [CTXD, 2 * DH], F32, isOutput=False)
    bctxk = nc.declare_dram_parameter("bctxk", [DH, 1], F32, isOutput=False)
    bctxv = nc.declare_dram_parameter("bctxv", [DH, 1], F32, isOutput=False)
    nullkt = nc.declare_dram_parameter("nullkt", [DH, 1], F32, isOutput=False)
    nullv = nc.declare_dram_parameter("nullv", [DH, 1], F32, isOutput=False)
    wout = nc.declare_dram_parameter("wout", [DH, HEADS * C], BF, isOutput=False)
    outg = nc.declare_dram_parameter("outg", [128, KT], F32, isOutput=False)
    y = nc.declare_dram_parameter("y", [C, NH], F32, isOutput=True)

    with tile.TileContext(nc) as tc, ExitStack() as ctx:
        pconst = ctx.enter_context(tc.tile_pool(name="const", bufs=1))
        pbig = ctx.enter_context(tc.tile_pool(name="big", bufs=1))

        ident = pconst.tile([128, 128], F32)
        make_identity(nc, ident[:])
        ident_bf = pconst.tile([128, 128], BF)
        make_identity(nc, ident_bf[:])
        ones_col = pconst.tile([128, 1], F32)
        nc.vector.memset(ones_col[:], 1.0)
        ones_blk = pconst.tile([128, 128], F32)
        nc.vector.memset(ones_blk[:], 1.0)
        eps_col = pconst.tile([128, 1], F32)
        nc.vector.memset(eps_col[:], EPS)

        x_sb = pbig.tile([128, KT * N], F32)         # kt-major; own rows first
        x_bf = pbig.tile([128, KT * N], BF)
        qT = pbig.tile([128, (HEADS // 2) * NH], BF)  # head-pair blocks
        kT2 = pbig.tile([128, JP], BF)               # rs-scaled keys, both halves
        v_sb = pbig.tile([128, JT * (DH + 1)], BF)   # per j-tile [128, 64+ones]
        projT = pbig.tile([128, KT * NH], F32)
        stats = pbig.tile([128, 40], F32)            # col jt: rs_j (v scaling)
        wout_sb = pbig.tile([64, HEADS * C], BF)
        outg_sb = pbig.tile([128, KT], F32)
        # per-token stat rows on partition 0: mu 0:N | rs N:2N
        # (LN2 reuses per cc: mu2 at cc*CH, rs2 at N+cc*CH, ex2 at 2N+cc*CH)
        rows = pbig.tile([1, 2 * N + NH], F32)
        rows_bf = pbig.tile([1, N], BF)
        R_RS, R_SC = N, 2 * N

        nc.sync.dma_start(wout_sb[:], wout[:, :])
        nc.sync.dma_start(outg_sb[:], outg[:, :])

        with tc.tile_pool(name="load", bufs=1) as pload, \
             tc.tile_pool(name="x2p", bufs=2) as px2, \
             tc.tile_pool(name="pss", bufs=2, space="PSUM") as pss:
            # setup psum tags: b1 [<=64,384]x2, bS [128,<=512]x4, bT [128,128]x2
            wq_sb = pload.tile([128, KT * C], BF)
            wkv_sb = pload.tile([128, KT * 2 * DH], BF)
            wctx_sb = pload.tile([128, CTXD], F32)
            negcq_sb = pload.tile([1, C], BF)
            ncsk_sb = pload.tile([1, DH], BF)
            ncsv_sb = pload.tile([1, DH], BF)
            bctxk_sb = pload.tile([DH, 1], F32)
            bctxv_sb = pload.tile([DH, 1], F32)
            vT = pload.tile([64, N], BF)
            ck_sb = pload.tile([64, CTXL], F32)
            cv_sb = pload.tile([64, CTXL + 1], F32)
            nullk_st = pload.tile([DH, 1], F32)
            nullv_st = pload.tile([DH, 1], F32)
            ctx_sb = pload.tile([CTXL, CTXD], F32)
            ctxnT = pload.tile([128, 6 * CTXL], F32)
            ex2 = pload.tile([1, N], F32)

            x_v = x_sb[:].rearrange("p (k n) -> p k n", k=KT)
            nc.sync.dma_start(x_v[:, :, 0:NH],
                              x_own[:].rearrange("(k p) n -> p k n", p=128))
            nc.sync.dma_start(x_v[:, :, NH:N],
                              x_oth[:].rearrange("(k p) n -> p k n", p=128))
            nc.sync.dma_start(wq_sb[:].rearrange("p (k n) -> p k n", k=KT),
                              wq[:].rearrange("(k p) n -> p k n", p=128))
            nc.sync.dma_start(wkv_sb[:].rearrange("p (k n) -> p k n", k=KT),
                              wkv[:].rearrange("(k p) n -> p k n", p=128))
            nc.sync.dma_start(wctx_sb[:].rearrange("p (k n) -> p k n", k=6),
                              wctx[:].rearrange("(k p) n -> p k n", p=128))
            nc.sync.dma_start(negcq_sb[:], negcq[:, :])
            nc.sync.dma_start(ncsk_sb[:], ncsk[:, :])
            nc.sync.dma_start(ncsv_sb[:], ncsv[:, :])
            nc.sync.dma_start(bctxk_sb[:], bctxk[:, :])
            nc.sync.dma_start(bctxv_sb[:], bctxv[:, :])
            nc.sync.dma_start(ctx_sb[:], ctxt[:, :])
            nc.sync.dma_start(nullk_st[:], nullkt[:, :])
            nc.sync.dma_start(nullv_st[:], nullv[:, :])

            for kt in range(KT):
                nc.vector.tensor_copy(x_bf[:, kt * N : (kt + 1) * N],
                                      x_sb[:, kt * N : (kt + 1) * N])

            # ---- LN1 stats (row form): mu, then rs = exp(-0.5 ln(var+eps)) ----
            ones_col_bf = pconst.tile([128, 1], BF)
            nc.vector.memset(ones_col_bf[:], 1.0)
            for ch in range(6):
                sl = slice(ch * 384, (ch + 1) * 384)
                ps_r1 = pss.tile([64, 384], F32, tag="b1")
                for kt in range(KT):
                    nc.tensor.matmul(
                        ps_r1[0:1, :], ones_col_bf[:],
                        x_bf[:, kt * N + ch * 384 : kt * N + (ch + 1) * 384],
                        start=(kt == 0), stop=(kt == KT - 1))
                nc.scalar.mul(rows[0:1, sl], ps_r1[0:1, :], 1.0 / C)
                nc.vector.tensor_copy(rows_bf[0:1, sl], rows[0:1, sl])
            for ch in range(6):
                x2 = px2.tile([128, KT * 384], BF, tag="x2")
                ps_r2 = pss.tile([64, 384], F32, tag="b1")
                for kt in range(KT):
                    xs = x_bf[:, kt * N + ch * 384 : kt * N + (ch + 1) * 384]
                    nc.vector.tensor_mul(x2[:, kt * 384 : (kt + 1) * 384], xs, xs)
                    nc.tensor.matmul(
                        ps_r2[0:1, :], ones_col_bf[:],
                        x2[:, kt * 384 : (kt + 1) * 384],
                        start=(kt == 0), stop=(kt == KT - 1))
                nc.scalar.mul(ex2[0:1, ch * 384 : (ch + 1) * 384],
                              ps_r2[0:1, :], 1.0 / C)
            for ch in range(6):
                a, b = R_RS + ch * 384, R_RS + (ch + 1) * 384
                mu = rows[0:1, ch * 384 : (ch + 1) * 384]
                nc.vector.tensor_mul(rows[0:1, a:b], mu, mu)
                nc.vector.tensor_sub(rows[0:1, a:b],
                                     ex2[0:1, ch * 384 : (ch + 1) * 384],
                                     rows[0:1, a:b])
                nc.scalar.activation(rows[0:1, a:b], rows[0:1, a:b], AF.Ln,
                                     bias=eps_col[0:1, :])
                nc.scalar.activation(rows[0:1, a:b], rows[0:1, a:b], AF.Exp,
                                     scale=-0.5)
            # rs as per-partition columns (v scaling)
            for jt in range(18):
                ps_c = pss.tile([128, 128], F32, tag="bT")
                nc.tensor.matmul(ps_c[:, 0:1],
                                 rows[0:1, R_RS + jt * 128 : R_RS + (jt + 1) * 128],
                                 ones_col[0:1, :])
                nc.vector.tensor_copy(stats[:, jt : jt + 1], ps_c[:, 0:1])

            # ---- context: LN (layout A, bn_stats) + k/v projection ----
            cstat = pload.tile([CTXL, 3, 6], F32)
            for sg in range(3):
                nc.vector.bn_stats(cstat[:, sg, :],
                                   ctx_sb[:, sg * 256 : (sg + 1) * 256])
            cmv = pload.tile([CTXL, 2], F32)
            nc.vector.bn_aggr(cmv[:], cstat[:])
            nc.scalar.activation(cmv[:, 1:2], cmv[:, 1:2], AF.Ln,
                                 bias=eps_col[0:CTXL, :])
            nc.scalar.activation(cmv[:, 1:2], cmv[:, 1:2], AF.Exp, scale=-0.5)
            nc.vector.tensor_scalar(
                out=ctx_sb[:], in0=ctx_sb[:],
                scalar1=cmv[:, 0:1], scalar2=cmv[:, 1:2],
                op0=ALU.subtract, op1=ALU.mult)
            for kt in range(6):
                ps_ct = pss.tile([128, 128], F32, tag="bT")
                nc.tensor.transpose(ps_ct[:, 0:CTXL],
                                    ctx_sb[:, kt * 128 : (kt + 1) * 128],
                                    ident[:CTXL, :CTXL])
                nc.vector.tensor_copy(ctxnT[:, kt * CTXL : (kt + 1) * CTXL],
                                      ps_ct[:, 0:CTXL])
            ps_ck = pss.tile([64, 384], F32, tag="b1")
            ps_cv = pss.tile([64, 384], F32, tag="b1")
            for kt in range(6):
                nc.tensor.matmul(ps_ck[:, 0:CTXL],
                                 wctx_sb[:, kt * 128 : kt * 128 + DH],
                                 ctxnT[:, kt * CTXL : (kt + 1) * CTXL],
                                 start=(kt == 0), stop=(kt == 5))
                nc.tensor.matmul(ps_cv[:, 0:CTXL],
                                 wctx_sb[:, kt * 128 + DH : (kt + 1) * 128],
                                 ctxnT[:, kt * CTXL : (kt + 1) * CTXL],
                                 start=(kt == 0), stop=(kt == 5))
            nc.vector.tensor_scalar_add(ck_sb[:], ps_ck[:, 0:CTXL], bctxk_sb[:])
            nc.vector.tensor_scalar_add(cv_sb[:, 0:CTXL], ps_cv[:, 0:CTXL],
                                        bctxv_sb[:])
            nc.vector.tensor_copy(cv_sb[:, CTXL : CTXL + 1], nullv_st[:])

            # ---- j-tile 18: [ctx 0:77 | null 77 | pad 78:128] ----
            VB = 18 * (DH + 1)
            nc.vector.memset(kT2[0:64, 18 * 128 : JP], 0.0)
            nc.vector.tensor_copy(kT2[0:64, 18 * 128 : 18 * 128 + CTXL], ck_sb[:])
            nc.vector.tensor_copy(kT2[0:64, 18 * 128 + CTXL : 18 * 128 + CTXL + 1],
                                  nullk_st[:])
            nc.vector.memset(v_sb[:, VB : VB + DH + 1], 0.0)
            ps_cvt = pss.tile([128, 128], F32, tag="bT")
            nc.tensor.transpose(ps_cvt[0 : CTXL + 1, 0:64], cv_sb[:],
                                ident[:64, :64])
            nc.vector.tensor_copy(v_sb[0 : CTXL + 1, VB : VB + DH],
                                  ps_cvt[0 : CTXL + 1, 0:64])
            nc.vector.memset(v_sb[0 : CTXL + 1, VB + DH : VB + DH + 1], 1.0)

            # ---- kv projection (all tokens; LN folded; kT rs-scaled) ----
            KVC = 384
            for ch in range(N // KVC):
                sl = slice(ch * KVC, (ch + 1) * KVC)
                ps_k = pss.tile([128, KVC], F32, tag="bS")
                ps_v = pss.tile([128, KVC], F32, tag="bS")
                for kt in range(KT):
                    xs = x_bf[:, kt * N + ch * KVC : kt * N + (ch + 1) * KVC]
                    nc.tensor.matmul(ps_k[0:64, :],
                                     wkv_sb[:, kt * 2 * DH : kt * 2 * DH + DH],
                                     xs, start=(kt == 0), stop=False)
                    nc.tensor.matmul(ps_v[0:64, :],
                                     wkv_sb[:, kt * 2 * DH + DH : (kt + 1) * 2 * DH],
                                     xs, start=(kt == 0), stop=False)
                nc.tensor.matmul(ps_k[0:64, :], ncsk_sb[:], rows_bf[0:1, sl],
                                 start=False, stop=True)
                nc.tensor.matmul(ps_v[0:64, :], ncsv_sb[:], rows_bf[0:1, sl],
                                 start=False, stop=True)
                ps_bc = pss.tile([128, KVC], F32, tag="bS")
                nc.tensor.matmul(ps_bc[0:64, :], ones_blk[0:1, 0:64],
                                 rows[0:1, R_RS + ch * KVC : R_RS + (ch + 1) * KVC])
                kk = px2.tile([64, KVC], F32, tag="kk")
                nc.vector.tensor_copy(kk[:], ps_k[0:64, :])
                nc.vector.tensor_mul(kT2[0:64, sl], kk[:], ps_bc[0:64, :])
                nc.vector.tensor_copy(vT[:, sl], ps_v[0:64, :])

            # ---- v tiles: transpose + rs scale + ones col ----
            for jt in range(18):
                ps_vt = pss.tile([128, 128], BF, tag="bT")
                nc.tensor.transpose(ps_vt[:, 0:64], vT[:, jt * 128 : (jt + 1) * 128],
                                    ident_bf[:64, :64])
                vb = jt * (DH + 1)
                nc.vector.tensor_scalar_mul(v_sb[:, vb : vb + DH], ps_vt[:, 0:64],
                                            stats[:, jt : jt + 1])
                nc.vector.memset(v_sb[:, vb + DH : vb + DH + 1], 1.0)

            # ---- duplicate kT to partitions 64:128 (sbuf->sbuf DMA) ----
            nc.sync.dma_start(kT2[64:128, :], kT2[0:64, :])

            # ---- q projection (head pairs; LN + 1/sqrt(dh) folded) ----
            for a0, ln in CHUNKS:
                sl = slice(a0, a0 + ln)
                ps_rs = pss.tile([128, 512], F32, tag="bS")
                nc.tensor.matmul(ps_rs[:, 0:ln], ones_blk[0:1, :],
                                 rows[0:1, R_RS + a0 : R_RS + a0 + ln])
                rs_b = px2.tile([128, 512], F32, tag="rsb")
                nc.vector.tensor_copy(rs_b[:, 0:ln], ps_rs[:, 0:ln])
                for hg in range(HEADS // 2):
                    ps_q = pss.tile([128, 512], F32, tag="bS")
                    for kt in range(KT):
                        nc.tensor.matmul(
                            ps_q[:, 0:ln],
                            wq_sb[:, kt * C + hg * 128 : kt * C + (hg + 1) * 128],
                            x_bf[:, kt * N + a0 : kt * N + a0 + ln],
                            start=(kt == 0), stop=False)
                    nc.tensor.matmul(ps_q[:, 0:ln],
                                     negcq_sb[0:1, hg * 128 : (hg + 1) * 128],
                                     rows_bf[0:1, sl], start=False, stop=True)
                    nc.vector.tensor_mul(
                        qT[:, hg * NH + a0 : hg * NH + a0 + ln],
                        ps_q[:, 0:ln], rs_b[:, 0:ln])

        # ========= attention + output + LN2 + residual, per chunk =========
        with tc.tile_pool(name="attn", bufs=4) as pattn, \
             tc.tile_pool(name="outp", bufs=9) as pout, \
             tc.tile_pool(name="recp", bufs=4) as prec, \
             tc.tile_pool(name="p2p", bufs=2) as pp2, \
             tc.tile_pool(name="yp", bufs=3) as pyt, \
             tc.tile_pool(name="psatt", bufs=2, space="PSUM") as psA, \
             tc.tile_pool(name="psacc", bufs=4, space="PSUM") as psB:
            psM = psB
            for cc, (a0, ln) in enumerate(CHUNKS):
                ots = []
                for hg in range(HEADS // 2):
                    po0 = psB.tile([128, 512], F32, tag="po")
                    po1 = psB.tile([128, 512], F32, tag="po")
                    po = [po0, po1]
                    q0 = qT[0:64, hg * NH + a0 : hg * NH + a0 + ln]
                    q1 = qT[64:128, hg * NH + a0 : hg * NH + a0 + ln]
                    ats = [None] * JT
                    # software pipeline: attn.v lags one j-tile behind exp;
                    # the two sims of a j-tile run row-packed (concurrent)
                    off1 = 512
                    for jt in range(JT):
                        ps_s = psA.tile([128, 1024], F32, tag="sim")
                        nc.tensor.matmul(ps_s[:, 0:ln],
                                         kT2[0:64, jt * 128 : (jt + 1) * 128],
                                         q0, start=True, stop=True)
                        nc.tensor.matmul(ps_s[:, off1 : off1 + ln],
                                         kT2[64:128, jt * 128 : (jt + 1) * 128],
                                         q1, start=True, stop=True)
                        at = pattn.tile([128, 1024], BF, tag="at")
                        if ln == 512:
                            nc.scalar.activation(at[:], ps_s[:], AF.Exp)
                        else:
                            nc.scalar.activation(at[:, 0:ln], ps_s[:, 0:ln], AF.Exp)
                            nc.scalar.activation(at[:, 512 : 512 + ln],
                                                 ps_s[:, 512 : 512 + ln], AF.Exp)
                        ats[jt] = at
                        if jt > 0:
                            j0 = jt - 1
                            vs = v_sb[:, j0 * (DH + 1) : (j0 + 1) * (DH + 1)]
                            nc.tensor.matmul(po[0][0:65, 0:ln], vs,
                                             ats[j0][:, 0:ln],
                                             start=(j0 == 0), stop=False)
                            nc.tensor.matmul(po[1][0:65, 0:ln], vs,
                                             ats[j0][:, off1 : off1 + ln],
                                             start=(j0 == 0), stop=False)
                            ats[j0] = None
                    j0 = JT - 1
                    vs = v_sb[:, j0 * (DH + 1) : (j0 + 1) * (DH + 1)]
                    nc.tensor.matmul(po[0][0:65, 0:ln], vs, ats[j0][:, 0:ln],
                                     start=False, stop=True)
                    nc.tensor.matmul(po[1][0:65, 0:ln], vs,
                                     ats[j0][:, off1 : off1 + ln],
                                     start=False, stop=True)
                    for i in range(2):
                        rec = prec.tile([65, 512], F32, tag="rec")
                        nc.vector.reciprocal(rec[64:65, 0:ln], po[i][64:65, 0:ln])
                        ps_rb = psM.tile([128, 512], F32, tag="po")
                        nc.tensor.matmul(ps_rb[0:64, 0:ln], ones_blk[64:65, 0:64],
                                         rec[64:65, 0:ln])
                        rb_sb = prec.tile([64, 512], F32, tag="rbs")
                        nc.vector.tensor_copy(rb_sb[:, 0:ln], ps_rb[0:64, 0:ln])
                        ot = pout.tile([64, 512], BF, tag="ot")
                        nc.vector.tensor_mul(ot[:, 0:ln], po[i][0:64, 0:ln],
                                             rb_sb[:, 0:ln])
                        ots.append(ot)
                for ct in range(KT):
                    ps_p = psA.tile([128, 1024], F32, tag="sim")
                    for h in range(HEADS):
                        nc.tensor.matmul(
                            ps_p[:, 0:ln],
                            wout_sb[:, h * C + ct * 128 : h * C + (ct + 1) * 128],
                            ots[h][:, 0:ln], start=(h == 0), stop=(h == HEADS - 1))
                    nc.vector.tensor_copy(
                        projT[:, ct * NH + a0 : ct * NH + a0 + ln],
                        ps_p[:, 0:ln])

                # ---- LN2 stats for this chunk ----
                ra, rb2 = R_RS + a0, R_RS + a0 + ln
                sca, scb = R_SC + a0, R_SC + a0 + ln
                ps_m2 = psM.tile([128, 512], F32, tag="po")
                for ct in range(KT):
                    nc.tensor.matmul(
                        ps_m2[0:1, 0:ln], ones_col[:],
                        projT[:, ct * NH + a0 : ct * NH + a0 + ln],
                        start=(ct == 0), stop=(ct == KT - 1))
                nc.scalar.mul(rows[0:1, a0 : a0 + ln], ps_m2[0:1, 0:ln], 1.0 / C)
                p2 = pp2.tile([128, KT * 512], F32, tag="p2")
                ps_q2 = psM.tile([128, 512], F32, tag="po")
                for ct in range(KT):
                    pslc = projT[:, ct * NH + a0 : ct * NH + a0 + ln]
                    nc.vector.tensor_mul(p2[:, ct * 512 : ct * 512 + ln], pslc, pslc)
                    nc.tensor.matmul(ps_q2[0:1, 0:ln], ones_col[:],
                                     p2[:, ct * 512 : ct * 512 + ln],
                                     start=(ct == 0), stop=(ct == KT - 1))
                nc.scalar.mul(rows[0:1, sca:scb], ps_q2[0:1, 0:ln], 1.0 / C)
                nc.vector.tensor_mul(rows[0:1, ra:rb2],
                                     rows[0:1, a0 : a0 + ln], rows[0:1, a0 : a0 + ln])
                nc.vector.tensor_sub(rows[0:1, ra:rb2],
                                     rows[0:1, sca:scb], rows[0:1, ra:rb2])
                nc.scalar.activation(rows[0:1, ra:rb2], rows[0:1, ra:rb2],
                                     AF.Ln, bias=eps_col[0:1, :])
                nc.scalar.activation(rows[0:1, ra:rb2], rows[0:1, ra:rb2],
                                     AF.Exp, scale=-0.5)

                # ---- y = x + outg * (projT - mu2) * rs2 ----
                ps_bm = psM.tile([128, 512], F32, tag="po")
                nc.tensor.matmul(ps_bm[:, 0:ln], ones_blk[0:1, :],
                                 rows[0:1, a0 : a0 + ln])
                ps_br = psM.tile([128, 512], F32, tag="po")
                nc.tensor.matmul(ps_br[:, 0:ln], ones_blk[0:1, :], rows[0:1, ra:rb2])
                for ct in range(KT):
                    yt = pyt.tile([128, 512], F32, tag="yt")
                    pslice = projT[:, ct * NH + a0 : ct * NH + a0 + ln]
                    nc.vector.tensor_sub(yt[:, 0:ln], pslice, ps_bm[:, 0:ln])
                    nc.vector.tensor_mul(yt[:, 0:ln], yt[:, 0:ln], ps_br[:, 0:ln])
                    nc.vector.tensor_scalar_mul(yt[:, 0:ln], yt[:, 0:ln],
                                                outg_sb[:, ct : ct + 1])
                    nc.vector.tensor_add(
                        yt[:, 0:ln], yt[:, 0:ln],
                        x_sb[:, ct * N + a0 : ct * N + a0 + ln])
                    nc.sync.dma_start(
                        y[ct * 128 : (ct + 1) * 128, a0 : a0 + ln], yt[:, 0:ln])
    _split_multi_waits(nc)
    return nc


def _prep_inputs(x, context, norm_gamma, null_kv, Wq, Wkv, ctx_ln_g, ctx_ln_b,
                 Wctx, bctx, Wout, out_ln_g):
    import ml_dtypes
    bf = ml_dtypes.bfloat16
    f = np.float32
    x = np.asarray(x, f).reshape(4, C, N)
    context = np.asarray(context, f)
    g = np.asarray(norm_gamma, f)
    scale = 1.0 / np.sqrt(DH)
    wq_h = (g[:, None] * np.asarray(Wq, f)) * scale
    negcq_h = -wq_h.sum(0, dtype=np.float64).astype(f)[None, :]
    wkv_h = g[:, None] * np.asarray(Wkv, f)
    ncsk_h = -wkv_h[:, :DH].sum(0, dtype=np.float64).astype(f)[None, :]
    ncsv_h = -wkv_h[:, DH:].sum(0, dtype=np.float64).astype(f)[None, :]
    wctx_h = np.asarray(ctx_ln_g, f)[:, None] * np.asarray(Wctx, f)
    bctx_h = (np.asarray(bctx, f) + np.asarray(ctx_ln_b, f) @ np.asarray(Wctx, f))
    null = np.asarray(null_kv, f)
    wout_b = np.concatenate(
        [np.asarray(Wout, f)[h * DH:(h + 1) * DH, :] for h in range(HEADS)], axis=1)
    outg_h = np.ascontiguousarray(np.asarray(out_ln_g, f).reshape(KT, 128).T)

    shared = {
        "wq": np.ascontiguousarray(wq_h).astype(bf),
        "negcq": negcq_h.astype(bf),
        "wkv": np.ascontiguousarray(wkv_h).astype(bf),
        "ncsk": ncsk_h.astype(bf), "ncsv": ncsv_h.astype(bf),
        "wctx": np.ascontiguousarray(wctx_h),
        "bctxk": np.ascontiguousarray(bctx_h[:DH, None]),
        "bctxv": np.ascontiguousarray(bctx_h[DH:, None]),
        "nullkt": np.ascontiguousarray(null[0][:, None]),
        "nullv": np.ascontiguousarray(null[1][:, None]),
        "wout": np.ascontiguousarray(wout_b).astype(bf),
        "outg": outg_h,
    }
    in_maps = []
    for core in range(8):
        b, half = core // 2, core % 2
        m = dict(shared)
        m["x_own"] = np.ascontiguousarray(x[b][:, half * NH : (half + 1) * NH])
        m["x_oth"] = np.ascontiguousarray(x[b][:, (1 - half) * NH : (2 - half) * NH])
        m["ctxt"] = np.ascontiguousarray(context[b])
        in_maps.append(m)
    return in_maps


def kernel(**inputs):
    from concourse.bass_utils import run_bass_kernel_spmd

    if "nc" not in _cached:
        _cached["nc"] = _build_bass()
    nc = _cached["nc"]
    in_maps = _prep_inputs(**inputs)
    kw = {}
    if PROFILE:
        import importlib.util

        if "antenv.axon_hooks" not in sys.modules:
            spec = importlib.util.spec_from_file_location(
                "antenv.axon_hooks", "/opt/trn_rl_repo/antenv/axon_hooks.py")
            m = importlib.util.module_from_spec(spec)
            spec.loader.exec_module(m)
            sys.modules["antenv.axon_hooks"] = m
            import antenv

            antenv.axon_hooks = m
        kw = dict(trace=True, tmpdir=PROFILE_DIR)
    res = run_bass_kernel_spmd(nc, in_maps, list(range(8)), **kw)
    _cached["last"] = res
    out = np.empty((4, C, N), np.float32)
    for core in range(8):
        b, half = core // 2, core % 2
        out[b][:, half * NH : (half + 1) * NH] = res.results[core]["y"]
    return out.reshape(4, C, 48, 48)


# revision 24
# speedup vs baseline: 1.3149x; 1.0716x over previous
"""Trainium2 Bass kernel for nn_Attention_LR_65249143160949 (cross-attention block).

Sharding: 8 cores = 4 batches x 2 token-halves (1152 tokens each). Each core
computes k/v for its whole batch (cheap MQA single head, duplicated within the
pair) and q/attention/output for its own tokens. The host permutes tokens so
each core's own rows come first -> identical SPMD program, no collectives.

On-chip layout: features on partitions, tokens on the free axis (matches the
channels-first HBM layout; no input transpose). LayerNorm is folded into the
projections: q = rs_i * (x @ Wq' - mu_i * colsum(Wq')), with Wq' pre-scaled on
the host; k/v analogous (rank-1 -colsum*mu matmul accumulated into the same
psum group). Attention runs in sim^T layout (keys on partitions, query tokens
on the free axis): kT is pre-scaled by rs_j so softmax is a plain exp; the
denominator comes free as a ones-column appended to v (row 64 of the out
psum); out^T columns are normalized by a PE-broadcast reciprocal row.
Per-token LN stats come from PE matmuls (ones as one operand), never from
cross-partition vector ops.

Precision: fp32 end-to-end math except the five big matmul groups
(q/kv/sim/attn.v/out-proj), whose operands are bf16 with fp32 PSUM
accumulation. LN statistics, softmax normalization, and the residual path
stay fp32.

Two walrus quirks are handled: every TPB instruction holds at most ONE sync
wait (extras are split onto same-engine NoOps by _split_multi_waits), and
custom DVE ops are unavailable (exact reciprocal is used).
"""

import sys

import numpy as np

if "/opt/trn_rl_repo" not in sys.path:
    sys.path.insert(0, "/opt/trn_rl_repo")

C = 512          # channels
N = 2304         # tokens per batch (48*48)
NH = 1152        # tokens per core
HEADS = 8
DH = 64
CTXL = 77
CTXD = 768
JT = 19          # j tiles of 128: 18 img + 1 (ctx 0:77 | null 77 | pad)
JP = JT * 128
CHUNKS = [(0, 512), (512, 512), (1024, 128)]  # (start, len) token chunks
NCH = len(CHUNKS)
KT = 4           # C / 128
EPS = 1e-5

PROFILE = False
PROFILE_DIR = None

_cached = {}


def _split_multi_waits(nc):
    """Walrus codegen supports one sync-wait per TPB instruction (the EVENTS
    struct has a single wait slot). Tile attaches several. Split the extras
    onto same-engine NoOps inserted just before each instruction."""
    import concourse.mybir as mybir

    n = 0
    for fn in nc.m.functions:
        for bb in fn.blocks:
            insts = bb.instructions
            i = 0
            while i < len(insts):
                ins = insts[i]
                si = getattr(ins, "sync_info", None)
                if si is not None and si.on_wait and len(si.on_wait) > 1:
                    waits = list(si.on_wait)
                    for w in waits[:-1]:
                        n += 1
                        nop = mybir.InstNoOp(name=f"WSPLIT-{n}", engine=ins.engine)
                        nop.sync_info = mybir.SyncInfo(on_wait=[w], on_update=[])
                        insts.insert(i, nop)
                        i += 1
                    ins.sync_info = mybir.SyncInfo(
                        on_wait=[waits[-1]], on_update=si.on_update)
                i += 1
    return n


def _build_bass():
    import concourse.bass as bass
    import concourse.mybir as mybir
    import concourse.tile as tile
    from concourse.masks import make_identity
    from contextlib import ExitStack

    F32 = mybir.dt.float32
    BF = mybir.dt.bfloat16
    AF = mybir.ActivationFunctionType
    ALU = mybir.AluOpType

    nc = bass.Bass()
    x_own = nc.declare_dram_parameter("x_own", [C, NH], F32, isOutput=False)
    x_oth = nc.declare_dram_parameter("x_oth", [C, NH], F32, isOutput=False)
    ctxt = nc.declare_dram_parameter("ctxt", [CTXL, CTXD], F32, isOutput=False)
    wq = nc.declare_dram_parameter("wq", [C, C], BF, isOutput=False)
    negcq = nc.declare_dram_parameter("negcq", [1, C], BF, isOutput=False)
    wkv = nc.declare_dram_parameter("wkv", [C, 2 * DH], BF, isOutput=False)
    ncsk = nc.declare_dram_parameter("ncsk", [1, DH], BF, isOutput=False)
    ncsv = nc.declare_dram_parameter("ncsv", [1, DH], BF, isOutput=False)
    wctx = nc.declare_dram_parameter("wctx", [CTXD, 2 * DH], F32, isOutput=False)
    bctxk = nc.declare_dram_parameter("bctxk", [DH, 1], F32, isOutput=False)
    bctxv = nc.declare_dram_parameter("bctxv", [DH, 1], F32, isOutput=False)
    nullkt = nc.declare_dram_parameter("nullkt", [DH, 1], F32, isOutput=False)
    nullv = nc.declare_dram_parameter("nullv", [DH, 1], F32, isOutput=False)
    wout = nc.declare_dram_parameter("wout", [DH, HEADS * C], BF, isOutput=False)
    outg = nc.declare_dram_parameter("outg", [128, KT], F32, isOutput=False)
    y = nc.declare_dram_parameter("y", [C, NH], F32, isOutput=True)

    with tile.TileContext(nc) as tc, ExitStack() as ctx:
        pconst = ctx.enter_context(tc.tile_pool(name="const", bufs=1))
        pbig = ctx.enter_context(tc.tile_pool(name="big", bufs=1))

        ident = pconst.tile([128, 128], F32)
        make_identity(nc, ident[:])
        ident_bf = pconst.tile([128, 128], BF)
        make_identity(nc, ident_bf[:])
        ones_col = pconst.tile([128, 1], F32)
        nc.vector.memset(ones_col[:], 1.0)
        ones_blk = pconst.tile([128, 128], F32)
        nc.vector.memset(ones_blk[:], 1.0)
        eps_col = pconst.tile([128, 1], F32)
        nc.vector.memset(eps_col[:], EPS)

        x_sb = pbig.tile([128, KT * N], F32)         # kt-major; own rows first
        x_bf = pbig.tile([128, KT * N], BF)
        qT = pbig.tile([128, (HEADS // 2) * NH], BF)  # head-pair blocks
        kT2 = pbig.tile([128, JP], BF)               # rs-scaled keys, both halves
        v_sb = pbig.tile([128, JT * (DH + 1)], BF)   # per j-tile [128, 64+ones]
        projT = pbig.tile([128, KT * NH], F32)
        stats = pbig.tile([128, 40], F32)            # col jt: rs_j (v scaling)
        wout_sb = pbig.tile([64, HEADS * C], BF)
        outg_sb = pbig.tile([128, KT], F32)
        # per-token stat rows on partition 0: mu 0:N | rs N:2N
        # (LN2 reuses per cc: mu2 at cc*CH, rs2 at N+cc*CH, ex2 at 2N+cc*CH)
        rows = pbig.tile([1, 2 * N + NH], F32)
        rows_bf = pbig.tile([1, N], BF)
        R_RS, R_SC = N, 2 * N

        nc.sync.dma_start(wout_sb[:], wout[:, :])
        nc.sync.dma_start(outg_sb[:], outg[:, :])

        with tc.tile_pool(name="load", bufs=1) as pload, \
             tc.tile_pool(name="x2p", bufs=2) as px2, \
             tc.tile_pool(name="pss", bufs=2, space="PSUM") as pss:
            # setup psum tags: b1 [<=64,384]x2, bS [128,<=512]x4, bT [128,128]x2
            wq_sb = pload.tile([128, KT * C], BF)
            wkv_sb = pload.tile([128, KT * 2 * DH], BF)
            wctx_sb = pload.tile([128, CTXD], F32)
            negcq_sb = pload.tile([1, C], BF)
            ncsk_sb = pload.tile([1, DH], BF)
            ncsv_sb = pload.tile([1, DH], BF)
            bctxk_sb = pload.tile([DH, 1], F32)
            bctxv_sb = pload.tile([DH, 1], F32)
            vT = pload.tile([64, N], BF)
            ck_sb = pload.tile([64, CTXL], F32)
            cv_sb = pload.tile([64, CTXL + 1], F32)
            nullk_st = pload.tile([DH, 1], F32)
            nullv_st = pload.tile([DH, 1], F32)
            ctx_sb = pload.tile([CTXL, CTXD], F32)
            ctxnT = pload.tile([128, 6 * CTXL], F32)
            ex2 = pload.tile([1, N], F32)

            x_v = x_sb[:].rearrange("p (k n) -> p k n", k=KT)
            nc.sync.dma_start(x_v[:, :, 0:NH],
                              x_own[:].rearrange("(k p) n -> p k n", p=128))
            nc.sync.dma_start(x_v[:, :, NH:N],
                              x_oth[:].rearrange("(k p) n -> p k n", p=128))
            nc.sync.dma_start(wq_sb[:].rearrange("p (k n) -> p k n", k=KT),
                              wq[:].rearrange("(k p) n -> p k n", p=128))
            nc.sync.dma_start(wkv_sb[:].rearrange("p (k n) -> p k n", k=KT),
                              wkv[:].rearrange("(k p) n -> p k n", p=128))
            nc.sync.dma_start(wctx_sb[:].rearrange("p (k n) -> p k n", k=6),
                              wctx[:].rearrange("(k p) n -> p k n", p=128))
            nc.sync.dma_start(negcq_sb[:], negcq[:, :])
            nc.sync.dma_start(ncsk_sb[:], ncsk[:, :])
            nc.sync.dma_start(ncsv_sb[:], ncsv[:, :])
            nc.sync.dma_start(bctxk_sb[:], bctxk[:, :])
            nc.sync.dma_start(bctxv_sb[:], bctxv[:, :])
            nc.sync.dma_start(ctx_sb[:], ctxt[:, :])
            nc.sync.dma_start(nullk_st[:], nullkt[:, :])
            nc.sync.dma_start(nullv_st[:], nullv[:, :])

            for kt in range(KT):
                nc.vector.tensor_copy(x_bf[:, kt * N : (kt + 1) * N],
                                      x_sb[:, kt * N : (kt + 1) * N])

            # ---- context: LN (layout A, bn_stats) + k/v projection ----
            cstat = pload.tile([CTXL, 3, 6], F32)
            for sg in range(3):
                nc.vector.bn_stats(cstat[:, sg, :],
                                   ctx_sb[:, sg * 256 : (sg + 1) * 256])
            cmv = pload.tile([CTXL, 2], F32)
            nc.vector.bn_aggr(cmv[:], cstat[:])
            nc.scalar.activation(cmv[:, 1:2], cmv[:, 1:2], AF.Ln,
                                 bias=eps_col[0:CTXL, :])
            nc.scalar.activation(cmv[:, 1:2], cmv[:, 1:2], AF.Exp, scale=-0.5)
            nc.vector.tensor_scalar(
                out=ctx_sb[:], in0=ctx_sb[:],
                scalar1=cmv[:, 0:1], scalar2=cmv[:, 1:2],
                op0=ALU.subtract, op1=ALU.mult)
            for kt in range(6):
                ps_ct = pss.tile([128, 128], F32, tag="bT")
                nc.tensor.transpose(ps_ct[:, 0:CTXL],
                                    ctx_sb[:, kt * 128 : (kt + 1) * 128],
                                    ident[:CTXL, :CTXL])
                nc.vector.tensor_copy(ctxnT[:, kt * CTXL : (kt + 1) * CTXL],
                                      ps_ct[:, 0:CTXL])
            ps_ck = pss.tile([64, 384], F32, tag="b1")
            ps_cv = pss.tile([64, 384], F32, tag="b1")
            for kt in range(6):
                nc.tensor.matmul(ps_ck[:, 0:CTXL],
                                 wctx_sb[:, kt * 128 : kt * 128 + DH],
                                 ctxnT[:, kt * CTXL : (kt + 1) * CTXL],
                                 start=(kt == 0), stop=(kt == 5))
                nc.tensor.matmul(ps_cv[:, 0:CTXL],
                                 wctx_sb[:, kt * 128 + DH : (kt + 1) * 128],
                                 ctxnT[:, kt * CTXL : (kt + 1) * CTXL],
                                 start=(kt == 0), stop=(kt == 5))
            nc.vector.tensor_scalar_add(ck_sb[:], ps_ck[:, 0:CTXL], bctxk_sb[:])
            nc.vector.tensor_scalar_add(cv_sb[:, 0:CTXL], ps_cv[:, 0:CTXL],
                                        bctxv_sb[:])
            nc.vector.tensor_copy(cv_sb[:, CTXL : CTXL + 1], nullv_st[:])

            # ---- LN1 stats (row form): mu, then rs = exp(-0.5 ln(var+eps)) ----
            ones_col_bf = pconst.tile([128, 1], BF)
            nc.vector.memset(ones_col_bf[:], 1.0)
            for ch in range(6):
                sl = slice(ch * 384, (ch + 1) * 384)
                ps_r1 = pss.tile([64, 384], F32, tag="b1")
                for kt in range(KT):
                    nc.tensor.matmul(
                        ps_r1[0:1, :], ones_col_bf[:],
                        x_bf[:, kt * N + ch * 384 : kt * N + (ch + 1) * 384],
                        start=(kt == 0), stop=(kt == KT - 1))
                nc.scalar.mul(rows[0:1, sl], ps_r1[0:1, :], 1.0 / C)
                nc.vector.tensor_copy(rows_bf[0:1, sl], rows[0:1, sl])
            for ch in range(6):
                x2 = px2.tile([128, KT * 384], BF, tag="x2")
                ps_r2 = pss.tile([64, 384], F32, tag="b1")
                for kt in range(KT):
                    xs = x_bf[:, kt * N + ch * 384 : kt * N + (ch + 1) * 384]
                    nc.vector.tensor_mul(x2[:, kt * 384 : (kt + 1) * 384], xs, xs)
                    nc.tensor.matmul(
                        ps_r2[0:1, :], ones_col_bf[:],
                        x2[:, kt * 384 : (kt + 1) * 384],
                        start=(kt == 0), stop=(kt == KT - 1))
                nc.scalar.mul(ex2[0:1, ch * 384 : (ch + 1) * 384],
                              ps_r2[0:1, :], 1.0 / C)
            for ch in range(6):
                a, b = R_RS + ch * 384, R_RS + (ch + 1) * 384
                mu = rows[0:1, ch * 384 : (ch + 1) * 384]
                nc.vector.tensor_mul(rows[0:1, a:b], mu, mu)
                nc.vector.tensor_sub(rows[0:1, a:b],
                                     ex2[0:1, ch * 384 : (ch + 1) * 384],
                                     rows[0:1, a:b])
                nc.scalar.activation(rows[0:1, a:b], rows[0:1, a:b], AF.Ln,
                                     bias=eps_col[0:1, :])
                nc.scalar.activation(rows[0:1, a:b], rows[0:1, a:b], AF.Exp,
                                     scale=-0.5)
            # rs as per-partition columns (v scaling)
            for jt in range(18):
                ps_c = pss.tile([128, 128], F32, tag="bT")
                nc.tensor.matmul(ps_c[:, 0:1],
                                 rows[0:1, R_RS + jt * 128 : R_RS + (jt + 1) * 128],
                                 ones_col[0:1, :])
                nc.vector.tensor_copy(stats[:, jt : jt + 1], ps_c[:, 0:1])

            # ---- j-tile 18: [ctx 0:77 | null 77 | pad 78:128] ----
            VB = 18 * (DH + 1)
            nc.vector.memset(kT2[0:64, 18 * 128 : JP], 0.0)
            nc.vector.tensor_copy(kT2[0:64, 18 * 128 : 18 * 128 + CTXL], ck_sb[:])
            nc.vector.tensor_copy(kT2[0:64, 18 * 128 + CTXL : 18 * 128 + CTXL + 1],
                                  nullk_st[:])
            nc.vector.memset(v_sb[:, VB : VB + DH + 1], 0.0)
            ps_cvt = pss.tile([128, 128], F32, tag="bT")
            nc.tensor.transpose(ps_cvt[0 : CTXL + 1, 0:64], cv_sb[:],
                                ident[:64, :64])
            nc.vector.tensor_copy(v_sb[0 : CTXL + 1, VB : VB + DH],
                                  ps_cvt[0 : CTXL + 1, 0:64])
            nc.vector.memset(v_sb[0 : CTXL + 1, VB + DH : VB + DH + 1], 1.0)

            # ---- kv projection (all tokens; LN folded; kT rs-scaled) ----
            KVC = 384
            for ch in range(N // KVC):
                sl = slice(ch * KVC, (ch + 1) * KVC)
                ps_k = pss.tile([128, KVC], F32, tag="bS")
                ps_v = pss.tile([128, KVC], F32, tag="bS")
                for kt in range(KT):
                    xs = x_bf[:, kt * N + ch * KVC : kt * N + (ch + 1) * KVC]
                    nc.tensor.matmul(ps_k[0:64, :],
                                     wkv_sb[:, kt * 2 * DH : kt * 2 * DH + DH],
                                     xs, start=(kt == 0), stop=False)
                    nc.tensor.matmul(ps_v[0:64, :],
                                     wkv_sb[:, kt * 2 * DH + DH : (kt + 1) * 2 * DH],
                                     xs, start=(kt == 0), stop=False)
                nc.tensor.matmul(ps_k[0:64, :], ncsk_sb[:], rows_bf[0:1, sl],
                                 start=False, stop=True)
                nc.tensor.matmul(ps_v[0:64, :], ncsv_sb[:], rows_bf[0:1, sl],
                                 start=False, stop=True)
                ps_bc = pss.tile([128, KVC], F32, tag="bS")
                nc.tensor.matmul(ps_bc[0:64, :], ones_blk[0:1, 0:64],
                                 rows[0:1, R_RS + ch * KVC : R_RS + (ch + 1) * KVC])
                kk = px2.tile([64, KVC], F32, tag="kk")
                nc.vector.tensor_copy(kk[:], ps_k[0:64, :])
                nc.vector.tensor_mul(kT2[0:64, sl], kk[:], ps_bc[0:64, :])
                nc.vector.tensor_copy(vT[:, sl], ps_v[0:64, :])

            # ---- v tiles: transpose + rs scale + ones col ----
            for jt in range(18):
                ps_vt = pss.tile([128, 128], BF, tag="bT")
                nc.tensor.transpose(ps_vt[:, 0:64], vT[:, jt * 128 : (jt + 1) * 128],
                                    ident_bf[:64, :64])
                vb = jt * (DH + 1)
                nc.vector.tensor_scalar_mul(v_sb[:, vb : vb + DH], ps_vt[:, 0:64],
                                            stats[:, jt : jt + 1])
                nc.vector.memset(v_sb[:, vb + DH : vb + DH + 1], 1.0)

            # ---- duplicate kT to partitions 64:128 (sbuf->sbuf DMA) ----
            nc.sync.dma_start(kT2[64:128, :], kT2[0:64, :])

            # ---- q projection (head pairs; LN + 1/sqrt(dh) folded) ----
            for a0, ln in CHUNKS:
                sl = slice(a0, a0 + ln)
                ps_rs = pss.tile([128, 512], F32, tag="bS")
                nc.tensor.matmul(ps_rs[:, 0:ln], ones_blk[0:1, :],
                                 rows[0:1, R_RS + a0 : R_RS + a0 + ln])
                rs_b = px2.tile([128, 512], F32, tag="rsb")
                nc.vector.tensor_copy(rs_b[:, 0:ln], ps_rs[:, 0:ln])
                for hg in range(HEADS // 2):
                    ps_q = pss.tile([128, 512], F32, tag="bS")
                    for kt in range(KT):
                        nc.tensor.matmul(
                            ps_q[:, 0:ln],
                            wq_sb[:, kt * C + hg * 128 : kt * C + (hg + 1) * 128],
                            x_bf[:, kt * N + a0 : kt * N + a0 + ln],
                            start=(kt == 0), stop=False)
                    nc.tensor.matmul(ps_q[:, 0:ln],
                                     negcq_sb[0:1, hg * 128 : (hg + 1) * 128],
                                     rows_bf[0:1, sl], start=False, stop=True)
                    nc.vector.tensor_mul(
                        qT[:, hg * NH + a0 : hg * NH + a0 + ln],
                        ps_q[:, 0:ln], rs_b[:, 0:ln])

        # ========= attention + output + LN2 + residual, per chunk =========
        # Per (chunk, head-pair): row-packed sims -> one exp -> attn.v pair
        # lagging one j-tile. Softmax normalization and the whole output tail
        # (out-proj, LN2, y) are DEFERRED one stage so slow reciprocals and
        # tail matmuls never head-of-line-block the in-order PE queue.
        with tc.tile_pool(name="attn", bufs=3) as pattn, \
             tc.tile_pool(name="outp", bufs=9) as pout, \
             tc.tile_pool(name="pocp", bufs=10) as ppoc, \
             tc.tile_pool(name="recp", bufs=10) as prec, \
             tc.tile_pool(name="rbsp", bufs=3) as prbs, \
             tc.tile_pool(name="p2p", bufs=2) as pp2, \
             tc.tile_pool(name="yp", bufs=3) as pyt, \
             tc.tile_pool(name="psatt", bufs=2, space="PSUM") as psA, \
             tc.tile_pool(name="psacc", bufs=4, space="PSUM") as psB:
            pending_tail = [None]

            def run_hg(hg, a0, ln, recs, pocs):
                po0 = psB.tile([128, 512], F32, tag="po")
                po1 = psB.tile([128, 512], F32, tag="po")
                po = [po0, po1]
                q0 = qT[0:64, hg * NH + a0 : hg * NH + a0 + ln]
                q1 = qT[64:128, hg * NH + a0 : hg * NH + a0 + ln]
                off1 = 512
                ats = [None] * JT
                for jt in range(JT):
                    ps_s = psA.tile([128, 1024], F32, tag="sim")
                    nc.tensor.matmul(ps_s[:, 0:ln],
                                     kT2[0:64, jt * 128 : (jt + 1) * 128],
                                     q0, start=True, stop=True)
                    nc.tensor.matmul(ps_s[:, off1 : off1 + ln],
                                     kT2[64:128, jt * 128 : (jt + 1) * 128],
                                     q1, start=True, stop=True)
                    at = pattn.tile([128, 1024], BF, tag="at")
                    if ln == 512:
                        nc.scalar.activation(at[:], ps_s[:], AF.Exp)
                    else:
                        nc.scalar.activation(at[:, 0:ln], ps_s[:, 0:ln], AF.Exp)
                        nc.scalar.activation(at[:, 512 : 512 + ln],
                                             ps_s[:, 512 : 512 + ln], AF.Exp)
                    ats[jt] = at
                    if jt > 0:
                        j0 = jt - 1
                        vs = v_sb[:, j0 * (DH + 1) : (j0 + 1) * (DH + 1)]
                        nc.tensor.matmul(po[0][0:65, 0:ln], vs, ats[j0][:, 0:ln],
                                         start=(j0 == 0), stop=False)
                        nc.tensor.matmul(po[1][0:65, 0:ln], vs,
                                         ats[j0][:, off1 : off1 + ln],
                                         start=(j0 == 0), stop=False)
                        ats[j0] = None
                j0 = JT - 1
                vs = v_sb[:, j0 * (DH + 1) : (j0 + 1) * (DH + 1)]
                nc.tensor.matmul(po[0][0:65, 0:ln], vs, ats[j0][:, 0:ln],
                                 start=False, stop=True)
                nc.tensor.matmul(po[1][0:65, 0:ln], vs,
                                 ats[j0][:, off1 : off1 + ln],
                                 start=False, stop=True)
                for i in range(2):
                    rec = prec.tile([65, 512], F32, tag="rec")
                    nc.vector.reciprocal(rec[64:65, 0:ln], po[i][64:65, 0:ln])
                    poc = ppoc.tile([64, 512], F32, tag="poc")
                    nc.vector.tensor_copy(poc[:, 0:ln], po[i][0:64, 0:ln])
                    recs.append(rec)
                    pocs.append(poc)

            def emit_tail(cc, a0, ln, recs, pocs):
                ots = []
                for h in range(HEADS):
                    ps_rb = psB.tile([128, 512], F32, tag="po")
                    nc.tensor.matmul(ps_rb[0:64, 0:ln], ones_blk[64:65, 0:64],
                                     recs[h][64:65, 0:ln])
                    rb_sb = prbs.tile([64, 512], F32, tag="rbs")
                    nc.vector.tensor_copy(rb_sb[:, 0:ln], ps_rb[0:64, 0:ln])
                    ot = pout.tile([64, 512], BF, tag="ot")
                    nc.vector.tensor_mul(ot[:, 0:ln], pocs[h][:, 0:ln],
                                         rb_sb[:, 0:ln])
                    ots.append(ot)
                for ct in range(KT):
                    ps_p = psA.tile([128, 1024], F32, tag="sim")
                    for h in range(HEADS):
                        nc.tensor.matmul(
                            ps_p[:, 0:ln],
                            wout_sb[:, h * C + ct * 128 : h * C + (ct + 1) * 128],
                            ots[h][:, 0:ln], start=(h == 0), stop=(h == HEADS - 1))
                    nc.vector.tensor_copy(
                        projT[:, ct * NH + a0 : ct * NH + a0 + ln], ps_p[:, 0:ln])

                ra, rb2 = R_RS + a0, R_RS + a0 + ln
                sca, scb = R_SC + a0, R_SC + a0 + ln
                ps_m2 = psB.tile([128, 512], F32, tag="po")
                for ct in range(KT):
                    nc.tensor.matmul(
                        ps_m2[0:1, 0:ln], ones_col[:],
                        projT[:, ct * NH + a0 : ct * NH + a0 + ln],
                        start=(ct == 0), stop=(ct == KT - 1))
                nc.scalar.mul(rows[0:1, a0 : a0 + ln], ps_m2[0:1, 0:ln], 1.0 / C)
                p2 = pp2.tile([128, KT * 512], F32, tag="p2")
                ps_q2 = psB.tile([128, 512], F32, tag="po")
                for ct in range(KT):
                    pslc = projT[:, ct * NH + a0 : ct * NH + a0 + ln]
                    nc.vector.tensor_mul(p2[:, ct * 512 : ct * 512 + ln], pslc, pslc)
                    nc.tensor.matmul(ps_q2[0:1, 0:ln], ones_col[:],
                                     p2[:, ct * 512 : ct * 512 + ln],
                                     start=(ct == 0), stop=(ct == KT - 1))
                nc.scalar.mul(rows[0:1, sca:scb], ps_q2[0:1, 0:ln], 1.0 / C)
                nc.vector.tensor_mul(rows[0:1, ra:rb2],
                                     rows[0:1, a0 : a0 + ln], rows[0:1, a0 : a0 + ln])
                nc.vector.tensor_sub(rows[0:1, ra:rb2],
                                     rows[0:1, sca:scb], rows[0:1, ra:rb2])
                nc.scalar.activation(rows[0:1, ra:rb2], rows[0:1, ra:rb2],
                                     AF.Ln, bias=eps_col[0:1, :])
                nc.scalar.activation(rows[0:1, ra:rb2], rows[0:1, ra:rb2],
                                     AF.Exp, scale=-0.5)
                ps_bm = psB.tile([128, 512], F32, tag="po")
                nc.tensor.matmul(ps_bm[:, 0:ln], ones_blk[0:1, :],
                                 rows[0:1, a0 : a0 + ln])
                ps_br = psB.tile([128, 512], F32, tag="po")
                nc.tensor.matmul(ps_br[:, 0:ln], ones_blk[0:1, :], rows[0:1, ra:rb2])
                for ct in range(KT):
                    yt = pyt.tile([128, 512], F32, tag="yt")
                    pslice = projT[:, ct * NH + a0 : ct * NH + a0 + ln]
                    nc.vector.tensor_sub(yt[:, 0:ln], pslice, ps_bm[:, 0:ln])
                    nc.vector.tensor_mul(yt[:, 0:ln], yt[:, 0:ln], ps_br[:, 0:ln])
                    nc.vector.tensor_scalar_mul(yt[:, 0:ln], yt[:, 0:ln],
                                                outg_sb[:, ct : ct + 1])
                    nc.vector.tensor_add(
                        yt[:, 0:ln], yt[:, 0:ln],
                        x_sb[:, ct * N + a0 : ct * N + a0 + ln])
                    nc.sync.dma_start(
                        y[ct * 128 : (ct + 1) * 128, a0 : a0 + ln], yt[:, 0:ln])

            pending = [None]
            for cc, (a0, ln) in enumerate(CHUNKS):
                recs, pocs = [], []
                for hg in range(HEADS // 2):
                    run_hg(hg, a0, ln, recs, pocs)
                    if hg == 0 and pending[0] is not None:
                        emit_tail(*pending[0])
                        pending[0] = None
                pending[0] = (cc, a0, ln, recs, pocs)
            emit_tail(*pending[0])
    _split_multi_waits(nc)
    return nc


def _prep_inputs(x, context, norm_gamma, null_kv, Wq, Wkv, ctx_ln_g, ctx_ln_b,
                 Wctx, bctx, Wout, out_ln_g):
    import ml_dtypes
    bf = ml_dtypes.bfloat16
    f = np.float32
    x = np.asarray(x, f).reshape(4, C, N)
    context = np.asarray(context, f)
    g = np.asarray(norm_gamma, f)
    scale = 1.0 / np.sqrt(DH)
    wq_h = (g[:, None] * np.asarray(Wq, f)) * scale
    negcq_h = -wq_h.sum(0, dtype=np.float64).astype(f)[None, :]
    wkv_h = g[:, None] * np.asarray(Wkv, f)
    ncsk_h = -wkv_h[:, :DH].sum(0, dtype=np.float64).astype(f)[None, :]
    ncsv_h = -wkv_h[:, DH:].sum(0, dtype=np.float64).astype(f)[None, :]
    wctx_h = np.asarray(ctx_ln_g, f)[:, None] * np.asarray(Wctx, f)
    bctx_h = (np.asarray(bctx, f) + np.asarray(ctx_ln_b, f) @ np.asarray(Wctx, f))
    null = np.asarray(null_kv, f)
    wout_b = np.concatenate(
        [np.asarray(Wout, f)[h * DH:(h + 1) * DH, :] for h in range(HEADS)], axis=1)
    outg_h = np.ascontiguousarray(np.asarray(out_ln_g, f).reshape(KT, 128).T)

    shared = {
        "wq": np.ascontiguousarray(wq_h).astype(bf),
        "negcq": negcq_h.astype(bf),
        "wkv": np.ascontiguousarray(wkv_h).astype(bf),
        "ncsk": ncsk_h.astype(bf), "ncsv": ncsv_h.astype(bf),
        "wctx": np.ascontiguousarray(wctx_h),
        "bctxk": np.ascontiguousarray(bctx_h[:DH, None]),
        "bctxv": np.ascontiguousarray(bctx_h[DH:, None]),
        "nullkt": np.ascontiguousarray(null[0][:, None]),
        "nullv": np.ascontiguousarray(null[1][:, None]),
        "wout": np.ascontiguousarray(wout_b).astype(bf),
        "outg": outg_h,
    }
    in_maps = []
    for core in range(8):
        b, half = core // 2, core % 2
        m = dict(shared)
        m["x_own"] = np.ascontiguousarray(x[b][:, half * NH : (half + 1) * NH])
        m["x_oth"] = np.ascontiguousarray(x[b][:, (1 - half) * NH : (2 - half) * NH])
        m["ctxt"] = np.ascontiguousarray(context[b])
        in_maps.append(m)
    return in_maps


def kernel(**inputs):
    from concourse.bass_utils import run_bass_kernel_spmd

    if "nc" not in _cached:
        _cached["nc"] = _build_bass()
    nc = _cached["nc"]
    in_maps = _prep_inputs(**inputs)
    kw = {}
    if PROFILE:
        import importlib.util

        if "antenv.axon_hooks" not in sys.modules:
            spec = importlib.util.spec_from_file_location(
                "antenv.axon_hooks", "/opt/trn_rl_repo/antenv/axon_hooks.py")
            m = importlib.util.module_from_spec(spec)
            spec.loader.exec_module(m)
            sys.modules["antenv.axon_hooks"] = m
            import antenv

            antenv.axon_hooks = m
        kw = dict(trace=True, tmpdir=PROFILE_DIR)
    res = run_bass_kernel_spmd(nc, in_maps, list(range(8)), **kw)
    _cached["last"] = res
    out = np.empty((4, C, N), np.float32)
    for core in range(8):
        b, half = core // 2, core % 2
        out[b][:, half * NH : (half + 1) * NH] = res.results[core]["y"]
    return out.reshape(4, C, 48, 48)


# revision 26
# speedup vs baseline: 1.3188x; 1.0029x over previous
"""Trainium2 Bass kernel for nn_Attention_LR_65249143160949 (cross-attention block).

Sharding: 8 cores = 4 batches x 2 token-halves (1152 tokens each). Each core
computes k/v for its whole batch (cheap MQA single head, duplicated within the
pair) and q/attention/output for its own tokens. The host permutes tokens so
each core's own rows come first -> identical SPMD program, no collectives.

On-chip layout: features on partitions, tokens on the free axis (matches the
channels-first HBM layout; no input transpose). LayerNorm is folded into the
projections: q = rs_i * (x @ Wq' - mu_i * colsum(Wq')), with Wq' pre-scaled on
the host; k/v analogous (rank-1 -colsum*mu matmul accumulated into the same
psum group). Attention runs in sim^T layout (keys on partitions, query tokens
on the free axis): kT is pre-scaled by rs_j so softmax is a plain exp; the
denominator comes free as a ones-column appended to v (row 64 of the out
psum); out^T columns are normalized by a PE-broadcast reciprocal row.
Per-token LN stats come from PE matmuls (ones as one operand), never from
cross-partition vector ops.

Precision: fp32 end-to-end math except the five big matmul groups
(q/kv/sim/attn.v/out-proj), whose operands are bf16 with fp32 PSUM
accumulation. LN statistics, softmax normalization, and the residual path
stay fp32.

Two walrus quirks are handled: every TPB instruction holds at most ONE sync
wait (extras are split onto same-engine NoOps by _split_multi_waits), and
custom DVE ops are unavailable (exact reciprocal is used).
"""

import sys

import numpy as np

if "/opt/trn_rl_repo" not in sys.path:
    sys.path.insert(0, "/opt/trn_rl_repo")

C = 512          # channels
N = 2304         # tokens per batch (48*48)
NH = 1152        # tokens per core
HEADS = 8
DH = 64
CTXL = 77
CTXD = 768
JT = 19          # j tiles of 128: 18 img + 1 (ctx 0:77 | null 77 | pad)
JP = JT * 128
CHUNKS = [(0, 512), (512, 512), (1024, 128)]  # (start, len) token chunks
NCH = len(CHUNKS)
KT = 4           # C / 128
EPS = 1e-5

PROFILE = False
PROFILE_DIR = None

_cached = {}


def _split_multi_waits(nc):
    """Walrus codegen supports one sync-wait per TPB instruction (the EVENTS
    struct has a single wait slot). Tile attaches several. Split the extras
    onto same-engine NoOps inserted just before each instruction."""
    import concourse.mybir as mybir

    n = 0
    for fn in nc.m.functions:
        for bb in fn.blocks:
            insts = bb.instructions
            i = 0
            while i < len(insts):
                ins = insts[i]
                si = getattr(ins, "sync_info", None)
                if si is not None and si.on_wait and len(si.on_wait) > 1:
                    waits = list(si.on_wait)
                    for w in waits[:-1]:
                        n += 1
                        nop = mybir.InstNoOp(name=f"WSPLIT-{n}", engine=ins.engine)
                        nop.sync_info = mybir.SyncInfo(on_wait=[w], on_update=[])
                        insts.insert(i, nop)
                        i += 1
                    ins.sync_info = mybir.SyncInfo(
                        on_wait=[waits[-1]], on_update=si.on_update)
                i += 1
    return n


def _build_bass():
    import concourse.bass as bass
    import concourse.mybir as mybir
    import concourse.tile as tile
    from concourse.masks import make_identity
    from contextlib import ExitStack

    F32 = mybir.dt.float32
    BF = mybir.dt.bfloat16
    AF = mybir.ActivationFunctionType
    ALU = mybir.AluOpType

    nc = bass.Bass()
    x_own = nc.declare_dram_parameter("x_own", [C, NH], F32, isOutput=False)
    x_oth = nc.declare_dram_parameter("x_oth", [C, NH], F32, isOutput=False)
    ctxt = nc.declare_dram_parameter("ctxt", [CTXL, CTXD], F32, isOutput=False)
    wq = nc.declare_dram_parameter("wq", [C, C], BF, isOutput=False)
    negcq = nc.declare_dram_parameter("negcq", [1, C], BF, isOutput=False)
    wkv = nc.declare_dram_parameter("wkv", [C, 2 * DH], BF, isOutput=False)
    ncsk = nc.declare_dram_parameter("ncsk", [1, DH], BF, isOutput=False)
    ncsv = nc.declare_dram_parameter("ncsv", [1, DH], BF, isOutput=False)
    wctx = nc.declare_dram_parameter("wctx", [CTXD, 2 * DH], F32, isOutput=False)
    bctxk = nc.declare_dram_parameter("bctxk", [DH, 1], F32, isOutput=False)
    bctxv = nc.declare_dram_parameter("bctxv", [DH, 1], F32, isOutput=False)
    nullkt = nc.declare_dram_parameter("nullkt", [DH, 1], F32, isOutput=False)
    nullv = nc.declare_dram_parameter("nullv", [DH, 1], F32, isOutput=False)
    wout = nc.declare_dram_parameter("wout", [DH, HEADS * C], BF, isOutput=False)
    outg = nc.declare_dram_parameter("outg", [128, KT], F32, isOutput=False)
    y = nc.declare_dram_parameter("y", [C, NH], F32, isOutput=True)

    with tile.TileContext(nc) as tc, ExitStack() as ctx:
        pconst = ctx.enter_context(tc.tile_pool(name="const", bufs=1))
        pbig = ctx.enter_context(tc.tile_pool(name="big", bufs=1))

        ident = pconst.tile([128, 128], F32)
        make_identity(nc, ident[:])
        ident_bf = pconst.tile([128, 128], BF)
        make_identity(nc, ident_bf[:])
        ones_col = pconst.tile([128, 1], F32)
        nc.vector.memset(ones_col[:], 1.0)
        ones_blk = pconst.tile([128, 128], F32)
        nc.vector.memset(ones_blk[:], 1.0)
        eps_col = pconst.tile([128, 1], F32)
        nc.vector.memset(eps_col[:], EPS)

        x_sb = pbig.tile([128, KT * N], F32)         # kt-major; own rows first
        x_bf = pbig.tile([128, KT * N], BF)
        qT = pbig.tile([128, (HEADS // 2) * NH], BF)  # head-pair blocks
        kT2 = pbig.tile([128, JP], BF)               # rs-scaled keys, both halves
        v_sb = pbig.tile([128, JT * (DH + 1)], BF)   # per j-tile [128, 64+ones]
        projT = pbig.tile([128, KT * NH], F32)
        stats = pbig.tile([128, 40], F32)            # col jt: rs_j (v scaling)
        wout_sb = pbig.tile([64, HEADS * C], BF)
        outg_sb = pbig.tile([128, KT], F32)
        # per-token stat rows on partition 0: mu 0:N | rs N:2N
        # (LN2 reuses per cc: mu2 at cc*CH, rs2 at N+cc*CH, ex2 at 2N+cc*CH)
        rows = pbig.tile([1, 2 * N + NH], F32)
        rows_bf = pbig.tile([1, N], BF)
        R_RS, R_SC = N, 2 * N

        nc.sync.dma_start(wout_sb[:], wout[:, :])
        nc.sync.dma_start(outg_sb[:], outg[:, :])

        with tc.tile_pool(name="load", bufs=1) as pload, \
             tc.tile_pool(name="x2p", bufs=2) as px2, \
             tc.tile_pool(name="pss", bufs=2, space="PSUM") as pss:
            # setup psum tags: b1 [<=64,384]x2, bS [128,<=512]x4, bT [128,128]x2
            wq_sb = pload.tile([128, KT * C], BF)
            wkv_sb = pload.tile([128, KT * 2 * DH], BF)
            wctx_sb = pload.tile([128, CTXD], F32)
            negcq_sb = pload.tile([1, C], BF)
            ncsk_sb = pload.tile([1, DH], BF)
            ncsv_sb = pload.tile([1, DH], BF)
            bctxk_sb = pload.tile([DH, 1], F32)
            bctxv_sb = pload.tile([DH, 1], F32)
            vT = pload.tile([64, N], BF)
            ck_sb = pload.tile([64, CTXL], F32)
            cv_sb = pload.tile([64, CTXL + 1], F32)
            nullk_st = pload.tile([DH, 1], F32)
            nullv_st = pload.tile([DH, 1], F32)
            ctx_sb = pload.tile([CTXL, CTXD], F32)
            ctxnT = pload.tile([128, 6 * CTXL], F32)
            ex2 = pload.tile([1, N], F32)

            x_v = x_sb[:].rearrange("p (k n) -> p k n", k=KT)
            nc.sync.dma_start(x_v[:, :, 0:NH],
                              x_own[:].rearrange("(k p) n -> p k n", p=128))
            nc.sync.dma_start(x_v[:, :, NH:N],
                              x_oth[:].rearrange("(k p) n -> p k n", p=128))
            nc.sync.dma_start(wq_sb[:].rearrange("p (k n) -> p k n", k=KT),
                              wq[:].rearrange("(k p) n -> p k n", p=128))
            nc.sync.dma_start(wkv_sb[:].rearrange("p (k n) -> p k n", k=KT),
                              wkv[:].rearrange("(k p) n -> p k n", p=128))
            nc.sync.dma_start(wctx_sb[:].rearrange("p (k n) -> p k n", k=6),
                              wctx[:].rearrange("(k p) n -> p k n", p=128))
            nc.sync.dma_start(negcq_sb[:], negcq[:, :])
            nc.sync.dma_start(ncsk_sb[:], ncsk[:, :])
            nc.sync.dma_start(ncsv_sb[:], ncsv[:, :])
            nc.sync.dma_start(bctxk_sb[:], bctxk[:, :])
            nc.sync.dma_start(bctxv_sb[:], bctxv[:, :])
            nc.sync.dma_start(ctx_sb[:], ctxt[:, :])
            nc.sync.dma_start(nullk_st[:], nullkt[:, :])
            nc.sync.dma_start(nullv_st[:], nullv[:, :])

            for kt in range(KT):
                nc.vector.tensor_copy(x_bf[:, kt * N : (kt + 1) * N],
                                      x_sb[:, kt * N : (kt + 1) * N])

            # ---- context: LN (layout A, bn_stats) + k/v projection ----
            cstat = pload.tile([CTXL, 3, 6], F32)
            for sg in range(3):
                nc.vector.bn_stats(cstat[:, sg, :],
                                   ctx_sb[:, sg * 256 : (sg + 1) * 256])
            cmv = pload.tile([CTXL, 2], F32)
            nc.vector.bn_aggr(cmv[:], cstat[:])
            nc.scalar.activation(cmv[:, 1:2], cmv[:, 1:2], AF.Ln,
                                 bias=eps_col[0:CTXL, :])
            nc.scalar.activation(cmv[:, 1:2], cmv[:, 1:2], AF.Exp, scale=-0.5)
            nc.vector.tensor_scalar(
                out=ctx_sb[:], in0=ctx_sb[:],
                scalar1=cmv[:, 0:1], scalar2=cmv[:, 1:2],
                op0=ALU.subtract, op1=ALU.mult)
            for kt in range(6):
                ps_ct = pss.tile([128, 128], F32, tag="bT")
                nc.tensor.transpose(ps_ct[:, 0:CTXL],
                                    ctx_sb[:, kt * 128 : (kt + 1) * 128],
                                    ident[:CTXL, :CTXL])
                nc.vector.tensor_copy(ctxnT[:, kt * CTXL : (kt + 1) * CTXL],
                                      ps_ct[:, 0:CTXL])
            ps_ck = pss.tile([64, 384], F32, tag="b1")
            ps_cv = pss.tile([64, 384], F32, tag="b1")
            for kt in range(6):
                nc.tensor.matmul(ps_ck[:, 0:CTXL],
                                 wctx_sb[:, kt * 128 : kt * 128 + DH],
                                 ctxnT[:, kt * CTXL : (kt + 1) * CTXL],
                                 start=(kt == 0), stop=(kt == 5))
                nc.tensor.matmul(ps_cv[:, 0:CTXL],
                                 wctx_sb[:, kt * 128 + DH : (kt + 1) * 128],
                                 ctxnT[:, kt * CTXL : (kt + 1) * CTXL],
                                 start=(kt == 0), stop=(kt == 5))
            nc.vector.tensor_scalar_add(ck_sb[:], ps_ck[:, 0:CTXL], bctxk_sb[:])
            nc.vector.tensor_scalar_add(cv_sb[:, 0:CTXL], ps_cv[:, 0:CTXL],
                                        bctxv_sb[:])
            nc.vector.tensor_copy(cv_sb[:, CTXL : CTXL + 1], nullv_st[:])

            # ---- LN1 stats (row form): mu, then rs = exp(-0.5 ln(var+eps)) ----
            ones_col_bf = pconst.tile([128, 1], BF)
            nc.vector.memset(ones_col_bf[:], 1.0)
            for ch in range(6):
                sl = slice(ch * 384, (ch + 1) * 384)
                ps_r1 = pss.tile([64, 384], F32, tag="b1")
                for kt in range(KT):
                    nc.tensor.matmul(
                        ps_r1[0:1, :], ones_col_bf[:],
                        x_bf[:, kt * N + ch * 384 : kt * N + (ch + 1) * 384],
                        start=(kt == 0), stop=(kt == KT - 1))
                nc.scalar.mul(rows[0:1, sl], ps_r1[0:1, :], 1.0 / C)
                nc.vector.tensor_copy(rows_bf[0:1, sl], rows[0:1, sl])
            for ch in range(6):
                x2 = px2.tile([128, KT * 384], BF, tag="x2")
                ps_r2 = pss.tile([64, 384], F32, tag="b1")
                for kt in range(KT):
                    xs = x_bf[:, kt * N + ch * 384 : kt * N + (ch + 1) * 384]
                    nc.vector.tensor_mul(x2[:, kt * 384 : (kt + 1) * 384], xs, xs)
                    nc.tensor.matmul(
                        ps_r2[0:1, :], ones_col_bf[:],
                        x2[:, kt * 384 : (kt + 1) * 384],
                        start=(kt == 0), stop=(kt == KT - 1))
                nc.scalar.mul(ex2[0:1, ch * 384 : (ch + 1) * 384],
                              ps_r2[0:1, :], 1.0 / C)
            for ch in range(6):
                a, b = R_RS + ch * 384, R_RS + (ch + 1) * 384
                mu = rows[0:1, ch * 384 : (ch + 1) * 384]
                nc.vector.tensor_mul(rows[0:1, a:b], mu, mu)
                nc.vector.tensor_sub(rows[0:1, a:b],
                                     ex2[0:1, ch * 384 : (ch + 1) * 384],
                                     rows[0:1, a:b])
                nc.scalar.activation(rows[0:1, a:b], rows[0:1, a:b], AF.Ln,
                                     bias=eps_col[0:1, :])
                nc.scalar.activation(rows[0:1, a:b], rows[0:1, a:b], AF.Exp,
                                     scale=-0.5)
            # rs as per-partition columns (v scaling)
            for jt in range(18):
                ps_c = pss.tile([128, 128], F32, tag="bT")
                nc.tensor.matmul(ps_c[:, 0:1],
                                 rows[0:1, R_RS + jt * 128 : R_RS + (jt + 1) * 128],
                                 ones_col[0:1, :])
                nc.vector.tensor_copy(stats[:, jt : jt + 1], ps_c[:, 0:1])

            # ---- j-tile 18: [ctx 0:77 | null 77 | pad 78:128] ----
            VB = 18 * (DH + 1)
            nc.vector.memset(kT2[0:64, 18 * 128 : JP], 0.0)
            nc.vector.tensor_copy(kT2[0:64, 18 * 128 : 18 * 128 + CTXL], ck_sb[:])
            nc.vector.tensor_copy(kT2[0:64, 18 * 128 + CTXL : 18 * 128 + CTXL + 1],
                                  nullk_st[:])
            nc.vector.memset(v_sb[:, VB : VB + DH + 1], 0.0)
            ps_cvt = pss.tile([128, 128], F32, tag="bT")
            nc.tensor.transpose(ps_cvt[0 : CTXL + 1, 0:64], cv_sb[:],
                                ident[:64, :64])
            nc.vector.tensor_copy(v_sb[0 : CTXL + 1, VB : VB + DH],
                                  ps_cvt[0 : CTXL + 1, 0:64])
            nc.vector.memset(v_sb[0 : CTXL + 1, VB + DH : VB + DH + 1], 1.0)

            # ---- kv projection (all tokens; LN folded; kT rs-scaled) ----
            KVC = 384
            for ch in range(N // KVC):
                sl = slice(ch * KVC, (ch + 1) * KVC)
                ps_k = pss.tile([128, KVC], F32, tag="bS")
                ps_v = pss.tile([128, KVC], F32, tag="bS")
                for kt in range(KT):
                    xs = x_bf[:, kt * N + ch * KVC : kt * N + (ch + 1) * KVC]
                    nc.tensor.matmul(ps_k[0:64, :],
                                     wkv_sb[:, kt * 2 * DH : kt * 2 * DH + DH],
                                     xs, start=(kt == 0), stop=False)
                    nc.tensor.matmul(ps_v[0:64, :],
                                     wkv_sb[:, kt * 2 * DH + DH : (kt + 1) * 2 * DH],
                                     xs, start=(kt == 0), stop=False)
                nc.tensor.matmul(ps_k[0:64, :], ncsk_sb[:], rows_bf[0:1, sl],
                                 start=False, stop=True)
                nc.tensor.matmul(ps_v[0:64, :], ncsv_sb[:], rows_bf[0:1, sl],
                                 start=False, stop=True)
                ps_bc = pss.tile([128, KVC], F32, tag="bS")
                nc.tensor.matmul(ps_bc[0:64, :], ones_blk[0:1, 0:64],
                                 rows[0:1, R_RS + ch * KVC : R_RS + (ch + 1) * KVC])
                kk = px2.tile([64, KVC], F32, tag="kk")
                nc.vector.tensor_copy(kk[:], ps_k[0:64, :])
                nc.vector.tensor_mul(kT2[0:64, sl], kk[:], ps_bc[0:64, :])
                nc.vector.tensor_copy(vT[:, sl], ps_v[0:64, :])

            # ---- v tiles: transpose + rs scale + ones col ----
            for jt in range(18):
                ps_vt = pss.tile([128, 128], BF, tag="bT")
                nc.tensor.transpose(ps_vt[:, 0:64], vT[:, jt * 128 : (jt + 1) * 128],
                                    ident_bf[:64, :64])
                vb = jt * (DH + 1)
                nc.vector.tensor_scalar_mul(v_sb[:, vb : vb + DH], ps_vt[:, 0:64],
                                            stats[:, jt : jt + 1])
                nc.vector.memset(v_sb[:, vb + DH : vb + DH + 1], 1.0)

            # ---- duplicate kT to partitions 64:128 (sbuf->sbuf DMA) ----
            nc.sync.dma_start(kT2[64:128, :], kT2[0:64, :])

            # ---- q projection (head pairs; LN + 1/sqrt(dh) folded) ----
            for a0, ln in CHUNKS:
                sl = slice(a0, a0 + ln)
                ps_rs = pss.tile([128, 512], F32, tag="bS")
                nc.tensor.matmul(ps_rs[:, 0:ln], ones_blk[0:1, :],
                                 rows[0:1, R_RS + a0 : R_RS + a0 + ln])
                rs_b = px2.tile([128, 512], F32, tag="rsb")
                nc.vector.tensor_copy(rs_b[:, 0:ln], ps_rs[:, 0:ln])
                for hg in range(HEADS // 2):
                    ps_q = pss.tile([128, 512], F32, tag="bS")
                    for kt in range(KT):
                        nc.tensor.matmul(
                            ps_q[:, 0:ln],
                            wq_sb[:, kt * C + hg * 128 : kt * C + (hg + 1) * 128],
                            x_bf[:, kt * N + a0 : kt * N + a0 + ln],
                            start=(kt == 0), stop=False)
                    nc.tensor.matmul(ps_q[:, 0:ln],
                                     negcq_sb[0:1, hg * 128 : (hg + 1) * 128],
                                     rows_bf[0:1, sl], start=False, stop=True)
                    nc.vector.tensor_mul(
                        qT[:, hg * NH + a0 : hg * NH + a0 + ln],
                        ps_q[:, 0:ln], rs_b[:, 0:ln])

        # ========= attention + output + LN2 + residual, per chunk =========
        # Per (chunk, head-pair): row-packed sims -> one exp -> attn.v pair
        # lagging one j-tile. Softmax normalization and the whole output tail
        # (out-proj, LN2, y) are DEFERRED one stage so slow reciprocals and
        # tail matmuls never head-of-line-block the in-order PE queue.
        with tc.tile_pool(name="attn", bufs=3) as pattn, \
             tc.tile_pool(name="outp", bufs=9) as pout, \
             tc.tile_pool(name="pocp", bufs=10) as ppoc, \
             tc.tile_pool(name="recp", bufs=10) as prec, \
             tc.tile_pool(name="rbsp", bufs=3) as prbs, \
             tc.tile_pool(name="p2p", bufs=2) as pp2, \
             tc.tile_pool(name="yp", bufs=3) as pyt, \
             tc.tile_pool(name="psatt", bufs=2, space="PSUM") as psA, \
             tc.tile_pool(name="psacc", bufs=4, space="PSUM") as psB:
            pending_tail = [None]

            def run_hg(hg, a0, ln, recs, pocs):
                po0 = psB.tile([128, 512], F32, tag="po")
                po1 = psB.tile([128, 512], F32, tag="po")
                po = [po0, po1]
                q0 = qT[0:64, hg * NH + a0 : hg * NH + a0 + ln]
                q1 = qT[64:128, hg * NH + a0 : hg * NH + a0 + ln]
                off1 = 512
                ats = [None] * JT
                for jt in range(JT):
                    ps_s = psA.tile([128, 1024], F32, tag="sim")
                    nc.tensor.matmul(ps_s[:, 0:ln],
                                     kT2[0:64, jt * 128 : (jt + 1) * 128],
                                     q0, start=True, stop=True)
                    nc.tensor.matmul(ps_s[:, off1 : off1 + ln],
                                     kT2[64:128, jt * 128 : (jt + 1) * 128],
                                     q1, start=True, stop=True)
                    at = pattn.tile([128, 1024], BF, tag="at")
                    if ln == 512:
                        nc.scalar.activation(at[:], ps_s[:], AF.Exp)
                    else:
                        nc.scalar.activation(at[:, 0:ln], ps_s[:, 0:ln], AF.Exp)
                        nc.scalar.activation(at[:, 512 : 512 + ln],
                                             ps_s[:, 512 : 512 + ln], AF.Exp)
                    ats[jt] = at
                    if jt > 0:
                        j0 = jt - 1
                        vs = v_sb[:, j0 * (DH + 1) : (j0 + 1) * (DH + 1)]
                        nc.tensor.matmul(po[0][0:65, 0:ln], vs, ats[j0][:, 0:ln],
                                         start=(j0 == 0), stop=False)
                        nc.tensor.matmul(po[1][0:65, 0:ln], vs,
                                         ats[j0][:, off1 : off1 + ln],
                                         start=(j0 == 0), stop=False)
                        ats[j0] = None
                j0 = JT - 1
                vs = v_sb[:, j0 * (DH + 1) : (j0 + 1) * (DH + 1)]
                nc.tensor.matmul(po[0][0:65, 0:ln], vs, ats[j0][:, 0:ln],
                                 start=False, stop=True)
                nc.tensor.matmul(po[1][0:65, 0:ln], vs,
                                 ats[j0][:, off1 : off1 + ln],
                                 start=False, stop=True)
                for i in range(2):
                    rec = prec.tile([65, 512], F32, tag="rec")
                    nc.vector.reciprocal(rec[64:65, 0:ln], po[i][64:65, 0:ln])
                    poc = ppoc.tile([64, 512], F32, tag="poc")
                    nc.vector.tensor_copy(poc[:, 0:ln], po[i][0:64, 0:ln])
                    recs.append(rec)
                    pocs.append(poc)

            def emit_tail(cc, a0, ln, recs, pocs):
                ots = []
                for h in range(HEADS):
                    ps_rb = psB.tile([128, 512], F32, tag="po")
                    nc.tensor.matmul(ps_rb[0:64, 0:ln], ones_blk[64:65, 0:64],
                                     recs[h][64:65, 0:ln])
                    rb_sb = prbs.tile([64, 512], F32, tag="rbs")
                    nc.vector.tensor_copy(rb_sb[:, 0:ln], ps_rb[0:64, 0:ln])
                    ot = pout.tile([64, 512], BF, tag="ot")
                    nc.vector.tensor_mul(ot[:, 0:ln], pocs[h][:, 0:ln],
                                         rb_sb[:, 0:ln])
                    ots.append(ot)
                for ct in range(KT):
                    ps_p = psA.tile([128, 1024], F32, tag="sim")
                    for h in range(HEADS):
                        nc.tensor.matmul(
                            ps_p[:, 0:ln],
                            wout_sb[:, h * C + ct * 128 : h * C + (ct + 1) * 128],
                            ots[h][:, 0:ln], start=(h == 0), stop=(h == HEADS - 1))
                    nc.vector.tensor_copy(
                        projT[:, ct * NH + a0 : ct * NH + a0 + ln], ps_p[:, 0:ln])

                ra, rb2 = R_RS + a0, R_RS + a0 + ln
                sca, scb = R_SC + a0, R_SC + a0 + ln
                ps_m2 = psB.tile([128, 512], F32, tag="po")
                for ct in range(KT):
                    nc.tensor.matmul(
                        ps_m2[0:1, 0:ln], ones_col[:],
                        projT[:, ct * NH + a0 : ct * NH + a0 + ln],
                        start=(ct == 0), stop=(ct == KT - 1))
                nc.scalar.mul(rows[0:1, a0 : a0 + ln], ps_m2[0:1, 0:ln], 1.0 / C)
                p2 = pp2.tile([128, KT * 512], F32, tag="p2")
                ps_q2 = psB.tile([128, 512], F32, tag="po")
                for ct in range(KT):
                    pslc = projT[:, ct * NH + a0 : ct * NH + a0 + ln]
                    nc.vector.tensor_mul(p2[:, ct * 512 : ct * 512 + ln], pslc, pslc)
                    nc.tensor.matmul(ps_q2[0:1, 0:ln], ones_col[:],
                                     p2[:, ct * 512 : ct * 512 + ln],
                                     start=(ct == 0), stop=(ct == KT - 1))
                nc.scalar.mul(rows[0:1, sca:scb], ps_q2[0:1, 0:ln], 1.0 / C)
                nc.vector.tensor_mul(rows[0:1, ra:rb2],
                                     rows[0:1, a0 : a0 + ln], rows[0:1, a0 : a0 + ln])
                nc.vector.tensor_sub(rows[0:1, ra:rb2],
                                     rows[0:1, sca:scb], rows[0:1, ra:rb2])
                nc.scalar.activation(rows[0:1, ra:rb2], rows[0:1, ra:rb2],
                                     AF.Ln, bias=eps_col[0:1, :])
                nc.scalar.activation(rows[0:1, ra:rb2], rows[0:1, ra:rb2],
                                     AF.Exp, scale=-0.5)
                ps_bm = psB.tile([128, 512], F32, tag="po")
                nc.tensor.matmul(ps_bm[:, 0:ln], ones_blk[0:1, :],
                                 rows[0:1, a0 : a0 + ln])
                ps_br = psB.tile([128, 512], F32, tag="po")
                nc.tensor.matmul(ps_br[:, 0:ln], ones_blk[0:1, :], rows[0:1, ra:rb2])
                for ct in range(KT):
                    yt = pyt.tile([128, 512], F32, tag="yt")
                    pslice = projT[:, ct * NH + a0 : ct * NH + a0 + ln]
                    nc.vector.tensor_sub(yt[:, 0:ln], pslice, ps_bm[:, 0:ln])
                    nc.vector.tensor_mul(yt[:, 0:ln], yt[:, 0:ln], ps_br[:, 0:ln])
                    nc.vector.tensor_scalar_mul(yt[:, 0:ln], yt[:, 0:ln],
                                                outg_sb[:, ct : ct + 1])
                    nc.vector.tensor_add(
                        yt[:, 0:ln], yt[:, 0:ln],
                        x_sb[:, ct * N + a0 : ct * N + a0 + ln])
                    nc.sync.dma_start(
                        y[ct * 128 : (ct + 1) * 128, a0 : a0 + ln], yt[:, 0:ln])

            pending = [None]
            for cc, (a0, ln) in enumerate(CHUNKS):
                recs, pocs = [], []
                for hg in range(HEADS // 2):
                    run_hg(hg, a0, ln, recs, pocs)
                    if hg == 0 and pending[0] is not None:
                        emit_tail(*pending[0])
                        pending[0] = None
                pending[0] = (cc, a0, ln, recs, pocs)
            emit_tail(*pending[0])
    _split_multi_waits(nc)
    return nc


def _prep_inputs(x, context, norm_gamma, null_kv, Wq, Wkv, ctx_ln_g, ctx_ln_b,
                 Wctx, bctx, Wout, out_ln_g):
    import ml_dtypes
    bf = ml_dtypes.bfloat16
    f = np.float32
    x = np.asarray(x, f).reshape(4, C, N)
    context = np.asarray(context, f)
    g = np.asarray(norm_gamma, f)
    scale = 1.0 / np.sqrt(DH)
    wq_h = (g[:, None] * np.asarray(Wq, f)) * scale
    negcq_h = -wq_h.sum(0, dtype=np.float64).astype(f)[None, :]
    wkv_h = g[:, None] * np.asarray(Wkv, f)
    ncsk_h = -wkv_h[:, :DH].sum(0, dtype=np.float64).astype(f)[None, :]
    ncsv_h = -wkv_h[:, DH:].sum(0, dtype=np.float64).astype(f)[None, :]
    wctx_h = np.asarray(ctx_ln_g, f)[:, None] * np.asarray(Wctx, f)
    bctx_h = (np.asarray(bctx, f) + np.asarray(ctx_ln_b, f) @ np.asarray(Wctx, f))
    null = np.asarray(null_kv, f)
    wout_b = np.concatenate(
        [np.asarray(Wout, f)[h * DH:(h + 1) * DH, :] for h in range(HEADS)], axis=1)
    outg_h = np.ascontiguousarray(np.asarray(out_ln_g, f).reshape(KT, 128).T)

    shared = {
        "wq": np.ascontiguousarray(wq_h).astype(bf),
        "negcq": negcq_h.astype(bf),
        "wkv": np.ascontiguousarray(wkv_h).astype(bf),
        "ncsk": ncsk_h.astype(bf), "ncsv": ncsv_h.astype(bf),
        "wctx": np.ascontiguousarray(wctx_h),
        "bctxk": np.ascontiguousarray(bctx_h[:DH, None]),
        "bctxv": np.ascontiguousarray(bctx_h[DH:, None]),
        "nullkt": np.ascontiguousarray(null[0][:, None]),
        "nullv": np.ascontiguousarray(null[1][:, None]),
        "wout": np.ascontiguousarray(wout_b).astype(bf),
        "outg": outg_h,
    }
    in_maps = []
    for core in range(8):
        b, half = core // 2, core % 2
        m = dict(shared)
        m["x_own"] = np.ascontiguousarray(x[b][:, half * NH : (half + 1) * NH])
        m["x_oth"] = np.ascontiguousarray(x[b][:, (1 - half) * NH : (2 - half) * NH])
        m["ctxt"] = np.ascontiguousarray(context[b])
        in_maps.append(m)
    return in_maps


_LDW_OPT = [False]


def _patch_ldw_opt():
    import concourse.bass_utils as bu
    if getattr(bu, "_ldwopt_patched", False):
        return
    orig = bu.run_command

    def run2(cmd, **kw):
        if _LDW_OPT[0]:
            cmd = [c.replace("--enable-ldw-opt=false", "--enable-ldw-opt=true")
                   for c in cmd]
        return orig(cmd, **kw)

    bu.run_command = run2
    bu._ldwopt_patched = True


def kernel(**inputs):
    from concourse.bass_utils import run_bass_kernel_spmd
    _patch_ldw_opt()

    if "nc" not in _cached:
        _cached["nc"] = _build_bass()
    nc = _cached["nc"]
    in_maps = _prep_inputs(**inputs)
    kw = {}
    if PROFILE:
        import importlib.util

        if "antenv.axon_hooks" not in sys.modules:
            spec = importlib.util.spec_from_file_location(
                "antenv.axon_hooks", "/opt/trn_rl_repo/antenv/axon_hooks.py")
            m = importlib.util.module_from_spec(spec)
            spec.loader.exec_module(m)
            sys.modules["antenv.axon_hooks"] = m
            import antenv

            antenv.axon_hooks = m
        kw = dict(trace=True, tmpdir=PROFILE_DIR)
    res = run_bass_kernel_spmd(nc, in_maps, list(range(8)), **kw)
    _cached["last"] = res
    out = np.empty((4, C, N), np.float32)
    for core in range(8):
        b, half = core // 2, core % 2
        out[b][:, half * NH : (half + 1) * NH] = res.results[core]["y"]
    return out.reshape(4, C, 48, 48)
